# revision 36
# baseline (speedup 1.0000x reference)
"""Trainium2 Bass kernel for a dense pre-LN transformer block.

B=4, T=1024, C=1024, H=16 heads (head_size 64).

Distribution over 8 NeuronCores, two SPMD launches with a free host-side
reduction between them:

  Launch A (attention): core c works on batch b=c//2 and head-half
  hh=c%2 (8 heads). It computes LN1 for its batch only, projects
  q/k/v for its heads, runs causal softmax(k@q^T)-attention in the
  transposed-scores layout, and multiplies by its slice of Wo rows,
  producing a PARTIAL projection [T, C] (f32) for its batch.

  Host: x2[b] = x[b] + part[2b] + part[2b+1] (+bo).

  Launch B (FFN): core c runs LN2 + W1/PReLU/W2 + residual on rows
  [512c, 512(c+1)) of x2.

Matmul dtype strategy: the PE cost depends only on the MOVING operand
dtype and its free size (1 cycle/row for bf16 at any N, f32r at N>=256).
Activations that move (hT, wo, w2 stream, e) stay f32r/bf16 chosen for
SBUF fit; weights that sit stationary are bf16 (0.4% quantization).
Accumulation is always f32 in PSUM.
"""

import os
from contextlib import ExitStack

import numpy as np

import concourse.bass as bass
import concourse.tile as tile
from concourse import bacc, mybir
from concourse.bass_utils import run_bass_kernel_spmd
from concourse.masks import make_identity

F32 = mybir.dt.float32
F32R = mybir.dt.float32r
BF16 = mybir.dt.bfloat16
AF = mybir.ActivationFunctionType
ALU = mybir.AluOpType

B, T, C, H, HS = 4, 1024, 1024, 16, 64
NCORES = 8
EPS = 1e-5
SCALE = float(C) ** -0.5  # folded into the softmax exp
NEG = -1e30

NTB = T // 128   # 8 token blocks per batch
NCC = C // 128   # 8 channel chunks
HPC = H // 2     # 8 heads per core
RPC = (B * T) // NCORES  # 512 rows per core in launch B
NRB = RPC // 128         # 4 row blocks
NHID = 4 * C // 128      # 32 hidden chunks


def _bank_chunks(lo, hi):
    """Split [lo, hi) at 512-column PSUM bank boundaries."""
    out = []
    o = lo
    while o < hi:
        n = min(512 - (o % 512), hi - o)
        out.append((o, o + n))
        o += n
    return out


# --------------------------------------------------------------------------
# kernel A: attention, one batch + 8 heads per core
# --------------------------------------------------------------------------

def _attn_body(ctx, tc, x, wq, wk, wv, wo, lnw, lnb, ones_dram, pout):
    nc = tc.nc
    general_ln = lnw is not None

    const = ctx.enter_context(tc.tile_pool(name="const", bufs=1))
    hTp = ctx.enter_context(tc.tile_pool(name="hTp", bufs=1))
    qTp = ctx.enter_context(tc.tile_pool(name="qTp", bufs=4))
    kTp = ctx.enter_context(tc.tile_pool(name="kTp", bufs=4))
    v2p = ctx.enter_context(tc.tile_pool(name="v2p", bufs=1))
    stat = ctx.enter_context(tc.tile_pool(name="stat", bufs=2))
    ep = ctx.enter_context(tc.tile_pool(name="ep", bufs=2))
    avp = ctx.enter_context(tc.tile_pool(name="avp", bufs=4))
    ctp = ctx.enter_context(tc.tile_pool(name="ctp", bufs=4))
    osp = ctx.enter_context(tc.tile_pool(name="osp", bufs=2))

    xp_cm = tc.tile_pool(name="xp", bufs=8)
    hp_cm = tc.tile_pool(name="hp", bufs=3)
    xp = xp_cm.__enter__()
    hp = hp_cm.__enter__()
    # x tiles first: these DMAs gate the LN1 critical path
    xts = []
    for i in range(NTB):
        xt = xp.tile([128, C], F32, tag="x", name=f"x_{i}")
        nc.sync.dma_start(out=xt, in_=x[i * 128:(i + 1) * 128, :])
        xts.append(xt)

    scratch = const.tile([128, 128], F32)
    make_identity(nc, scratch)
    ident = const.tile([128, 128], BF16)
    nc.vector.tensor_copy(out=ident, in_=scratch)
    # transposed causal mask for diagonal blocks: keep s<=t (cols>=rows)
    trilT = const.tile([128, 128], F32)
    nc.gpsimd.memset(trilT, 0.0)
    nc.gpsimd.affine_select(
        out=trilT, in_=trilT, compare_op=ALU.is_ge, fill=NEG, base=0,
        pattern=[[1, 128]], channel_multiplier=-1)
    eps_t = const.tile([128, 1], F32)
    nc.vector.memset(eps_t, EPS)
    ones64 = const.tile([1, 64], BF16)
    nc.sync.dma_start(out=ones64, in_=ones_dram)
    if general_ln:
        lnw_bc = const.tile([128, C], F32, tag="lnw")
        lnb_bc = const.tile([128, C], F32, tag="lnb")
        nc.sync.dma_start(
            out=lnw_bc,
            in_=bass.AP(tensor=lnw.tensor, offset=lnw.offset,
                        ap=[[0, 128]] + list(lnw.ap)))
        nc.sync.dma_start(
            out=lnb_bc,
            in_=bass.AP(tensor=lnb.tensor, offset=lnb.offset,
                        ap=[[0, 128]] + list(lnb.ap)))

    # weights resident in SBUF (after x: LN1 must not wait behind these)
    wq_sb = const.tile([128, NCC, 512], BF16, tag="wq")
    wk_sb = const.tile([128, NCC, 512], BF16, tag="wk")
    wv_sb = const.tile([128, NCC, 512], BF16, tag="wv")
    wo_sb = const.tile([128, 4, C], BF16, tag="wo")
    nc.sync.dma_start(out=wq_sb, in_=wq)
    nc.sync.dma_start(out=wk_sb, in_=wk)
    nc.sync.dma_start(out=wv_sb, in_=wv)
    nc.sync.dma_start(out=wo_sb, in_=wo)

    hT = hTp.tile([128, NCC, T], BF16, tag="hT")

    # ---- Phase A: LN1 (own batch only) + transpose, 2 groups of 4 ----
    with tc.tile_pool(name="pst", bufs=2, space="PSUM") as PST:
        for grp in range(2):
            mvs = stat.tile([128, 4, 2], F32, tag="mvs", name=f"mvs_{grp}")
            rstd = stat.tile([128, 4], F32, tag="rstd", name=f"rstd_{grp}")
            lnv = stat.tile([128, 4], F32, tag="lnv", name=f"lnv_{grp}")
            for j in range(4):
                i = grp * 4 + j
                st = stat.tile([128, 2, 6], F32, tag="bn", name=f"bn_{i}")
                for k in range(2):
                    nc.vector.bn_stats(out=st[:, k, :],
                                       in_=xts[i][:, k * 512:(k + 1) * 512])
                nc.vector.bn_aggr(out=mvs[:, j, :], in_=st)
            nc.scalar.activation(out=lnv, in_=mvs[:, :, 1], func=AF.Ln,
                                 bias=eps_t)
            nc.scalar.activation(out=rstd, in_=lnv, func=AF.Exp, scale=-0.5)
            for j in range(4):
                i = grp * 4 + j
                ht = hp.tile([128, C], BF16, tag="h", name=f"h_{i}")
                nc.vector.tensor_scalar(
                    out=ht, in0=xts[i], scalar1=mvs[:, j, 0:1],
                    scalar2=rstd[:, j:j + 1], op0=ALU.subtract, op1=ALU.mult)
                if general_ln:
                    nc.vector.tensor_mul(out=ht, in0=ht, in1=lnw_bc)
                    nc.vector.tensor_add(out=ht, in0=ht, in1=lnb_bc)
                for g in range(2):
                    pt = PST.tile([128, 512], BF16, tag="tr",
                                  name=f"pt_{i}_{g}")
                    for c in range(4):
                        cc = g * 4 + c
                        nc.tensor.transpose(
                            pt[:, c * 128:(c + 1) * 128],
                            ht[:, cc * 128:(cc + 1) * 128], ident)
                    for c in range(4):
                        cc = g * 4 + c
                        nc.vector.tensor_copy(
                            out=hT[:, cc, i * 128:(i + 1) * 128],
                            in_=pt[:, c * 128:(c + 1) * 128])
    hp_cm.__exit__(None, None, None)
    xp_cm.__exit__(None, None, None)
    dnp = ctx.enter_context(tc.tile_pool(name="dnp", bufs=8))
    lnp = ctx.enter_context(tc.tile_pool(name="lnp", bufs=8))
    recp = ctx.enter_context(tc.tile_pool(name="recp", bufs=8))

    # ---- Phase B+C+D interleaved: qkv, scores+exp, av ----
    qT = [qTp.tile([128, T], BF16, tag="qT", name=f"qT_{m}")
          for m in range(4)]
    kT = [kTp.tile([128, T], BF16, tag="kT", name=f"kT_{m}")
          for m in range(4)]
    # v2[tok, sc, h, 0:64] = v; col 64 = ones (softmax denominator)
    v2 = v2p.tile([128, NTB, HPC, 66], BF16, tag="v2")
    nc.vector.memset(v2[:, :, :, 64:65], 1.0)
    nc.vector.memset(v2[:, :, :, 65:66], 0.0)
    cat = [ctp.tile([128, T], BF16, tag="cat", name=f"cat_{m}")
           for m in range(4)]
    # per-head softmax denominator rows and their reciprocals
    dens = [dnp.tile([1, T], BF16, tag="den", name=f"den_{h}")
            for h in range(HPC)]
    lnds = [lnp.tile([1, T], F32, tag="lnd", name=f"lnd_{h}")
            for h in range(HPC)]
    recs = [recp.tile([1, T], BF16, tag="rec", name=f"rec_{h}")
            for h in range(HPC)]
    pav_sbs = [avp.tile([128, T], F32R, tag="avsb", name=f"avsb_{m}")
               for m in range(4)]

    # PSUM budget: PSQ 2x[128,512]=2 banks, PSS 2x[128,<=512]=2,
    # PSA 2x[66,1024]=4  -> 8 banks total.
    with tc.tile_pool(name="psq", bufs=2, space="PSUM") as PSQ, \
         tc.tile_pool(name="pss", bufs=2, space="PSUM") as PSS, \
         tc.tile_pool(name="psa", bufs=2, space="PSUM") as PSA:

        def qkv_m(m):
            for th in range(2):
                tsl = slice(th * 512, (th + 1) * 512)
                pq = PSQ.tile([128, 512], F32, tag="mm", name=f"pq_{m}_{th}")
                for cc in range(NCC):
                    nc.tensor.matmul(pq, wq_sb[:, cc, m * 128:(m + 1) * 128],
                                     hT[:, cc, tsl],
                                     start=(cc == 0), stop=(cc == NCC - 1))
                nc.vector.tensor_copy(out=qT[m][:, tsl], in_=pq)
                pk = PSQ.tile([128, 512], F32, tag="mm", name=f"pk_{m}_{th}")
                for cc in range(NCC):
                    nc.tensor.matmul(pk, wk_sb[:, cc, m * 128:(m + 1) * 128],
                                     hT[:, cc, tsl],
                                     start=(cc == 0), stop=(cc == NCC - 1))
                nc.vector.tensor_copy(out=kT[m][:, tsl], in_=pk)

        def v_tb(tb):
            pv = PSQ.tile([128, 512], F32, tag="mm", name=f"pv_{tb}")
            for cc in range(NCC):
                nc.tensor.matmul(pv, hT[:, cc, tb * 128:(tb + 1) * 128],
                                 wv_sb[:, cc, :],
                                 start=(cc == 0), stop=(cc == NCC - 1))
            nc.vector.tensor_copy(
                out=v2[:, tb, :, 0:64],
                in_=bass.AP(tensor=pv.tensor, offset=pv.offset,
                            ap=list(pv.ap[:1]) + [[64, HPC], [1, 64]]))

        def scores_h(h):
            """scoresT + exp for head h; returns e tiles per sc."""
            m, ho = h // 2, (h % 2) * 64
            qh = qT[m][ho:ho + 64, :]
            kh = kT[m][ho:ho + 64, :]
            es = []
            for sc in range(NTB):
                W = T - sc * 128
                e = ep.tile([128, W], BF16, tag=f"e{sc}", name=f"e_{h}_{sc}")
                for (o0, o1) in _bank_chunks(0, W):
                    ps = PSS.tile([128, o1 - o0], F32, tag="sc",
                                  name=f"ps_{h}_{sc}_{o0}")
                    nc.tensor.matmul(
                        ps,
                        qh[:, sc * 128:(sc + 1) * 128],
                        kh[:, sc * 128 + o0:sc * 128 + o1],
                        start=True, stop=True)
                    if o0 == 0:
                        nc.vector.tensor_add(out=ps[:, 0:128],
                                             in0=ps[:, 0:128], in1=trilT)
                    nc.scalar.activation(out=e[:, o0:o1], in_=ps,
                                         func=AF.Exp, scale=SCALE)
                es.append(e)
            return es

        def av_h(h, es):
            """av for head h into pav_sb half + stash denominator row."""
            pav = PSA.tile([66, T], F32, tag="av", name=f"pav_{h}")
            for sc in range(NTB):
                lo = sc * 128
                for (o0, o1) in _bank_chunks(lo, T):
                    nc.tensor.matmul(
                        pav[:, o0:o1], v2[:, sc, h, :],
                        es[sc][:, o0 - lo:o1 - lo],
                        start=(sc == 0), stop=(sc == NTB - 1),
                        skip_group_check=True)
            ho = (h % 2) * 64
            nc.vector.tensor_copy(out=pav_sbs[h // 2][ho:ho + 64, :],
                                  in_=pav[0:64, :])
            nc.scalar.activation(out=dens[h], in_=pav[64:65, :],
                                 func=AF.Identity)

        es0 = qkv_m(0) or scores_h(0)
        qkv_m(1)
        for tb in range(NTB):
            v_tb(tb)
        es1 = scores_h(1)
        qkv_m(2)
        es2 = scores_h(2)
        qkv_m(3)

        def recip_grp(grp):
            with nc.allow_low_precision(reason="softmax denominator recip"):
                for i in range(4):
                    nc.scalar.activation(out=lnds[grp * 4 + i],
                                         in_=dens[grp * 4 + i], func=AF.Ln)
                for i in range(4):
                    nc.scalar.activation(out=recs[grp * 4 + i],
                                         in_=lnds[grp * 4 + i],
                                         func=AF.Exp, scale=-1.0)

        es = {0: es0, 1: es1, 2: es2}
        for h in range(HPC):
            if h + 3 < HPC:
                es[h + 3] = scores_h(h + 3)
            av_h(h, es.pop(h))
            if h == 3 or h == 7:
                recip_grp(h // 4)

    # ---- Phase E: 1/den via exp(-ln(den)), normalize, Wo projection ----
    with tc.tile_pool(name="psb", bufs=2, space="PSUM") as PSB, \
         tc.tile_pool(name="psp", bufs=2, space="PSUM") as PSP:
        for m in range(4):
            # partition-broadcast each head's 1/den row via K=1 PE matmul
            prec = PSB.tile([128, T], F32, tag="prec", name=f"prec_{m}")
            for hh in range(2):
                for co in range(2):
                    nc.tensor.matmul(
                        prec[hh * 64:(hh + 1) * 64, co * 512:(co + 1) * 512],
                        ones64,
                        recs[2 * m + hh][:, co * 512:(co + 1) * 512],
                        start=True, stop=True)
            nc.vector.tensor_mul(out=cat[m], in0=pav_sbs[m], in1=prec)

        for tb in range(NTB):
            pp = PSP.tile([128, C], F32, tag="pp", name=f"pp_{tb}")
            for m in range(4):
                for co in range(2):
                    nc.tensor.matmul(
                        pp[:, co * 512:(co + 1) * 512],
                        cat[m][:, tb * 128:(tb + 1) * 128],
                        wo_sb[:, m, co * 512:(co + 1) * 512],
                        start=(m == 0), stop=(m == 3))
            o_sb = osp.tile([128, C], BF16, tag="o", name=f"o_{tb}")
            nc.vector.tensor_copy(out=o_sb, in_=pp)
            nc.sync.dma_start(out=pout[tb * 128:(tb + 1) * 128, :], in_=o_sb)


def _build_attn(general_ln: bool):
    nc = bacc.Bacc("TRN2", target_bir_lowering=False, debug=False)
    x = nc.dram_tensor("x", [T, C], F32, kind="ExternalInput").ap()
    wq = nc.dram_tensor("wq", [128, NCC, 512], BF16, kind="ExternalInput").ap()
    wk = nc.dram_tensor("wk", [128, NCC, 512], BF16, kind="ExternalInput").ap()
    wv = nc.dram_tensor("wv", [128, NCC, 512], BF16, kind="ExternalInput").ap()
    wo = nc.dram_tensor("wo", [128, 4, C], BF16, kind="ExternalInput").ap()
    lnw = lnb = None
    if general_ln:
        lnw = nc.dram_tensor("lnw", [C], F32, kind="ExternalInput").ap()
        lnb = nc.dram_tensor("lnb", [C], F32, kind="ExternalInput").ap()
    ones_dram = nc.dram_tensor("ones", [1, 64], BF16,
                               kind="ExternalInput").ap()
    pout = nc.dram_tensor("pout", [T, C], BF16, kind="ExternalOutput").ap()
    with tile.TileContext(nc) as tc:
        with ExitStack() as ctx:
            _attn_body(ctx, tc, x, wq, wk, wv, wo, lnw, lnb, ones_dram, pout)
    nc.compile()
    return nc


# --------------------------------------------------------------------------
# kernel B: FFN, 512 rows per core
# --------------------------------------------------------------------------

def _ffn_body(ctx, tc, x2, w1, w2, b1, lnw, lnb, alpha, out):
    nc = tc.nc
    general_ln = lnw is not None

    const = ctx.enter_context(tc.tile_pool(name="const", bufs=1))
    xp = ctx.enter_context(tc.tile_pool(name="xp", bufs=NRB))
    # x2 tiles first: these DMAs gate the LN2 critical path
    x2ts = []
    for r in range(NRB):
        xt = xp.tile([128, C], F32, tag="x", name=f"x_{r}")
        nc.sync.dma_start(out=xt, in_=x2[r * 128:(r + 1) * 128, :])
        x2ts.append(xt)
    scratch = const.tile([128, 128], F32)
    make_identity(nc, scratch)
    ident = const.tile([128, 128], BF16)
    nc.vector.tensor_copy(out=ident, in_=scratch)
    eps_t = const.tile([128, 1], F32)
    nc.vector.memset(eps_t, EPS)
    if general_ln:
        lnw_bc = const.tile([128, C], F32, tag="lnw")
        lnb_bc = const.tile([128, C], F32, tag="lnb")
        nc.sync.dma_start(
            out=lnw_bc,
            in_=bass.AP(tensor=lnw.tensor, offset=lnw.offset,
                        ap=[[0, 128]] + list(lnw.ap)))
        nc.sync.dma_start(
            out=lnb_bc,
            in_=bass.AP(tensor=lnb.tensor, offset=lnb.offset,
                        ap=[[0, 128]] + list(lnb.ap)))
    b1_sb = None
    if b1 is not None:
        b1_sb = const.tile([128, NHID], F32, tag="b1")
        nc.sync.dma_start(out=b1_sb, in_=b1.rearrange("(h p) -> p h", p=128))

    # weights: big resident tiles, streamed in chunks of 8 hidden blocks
    w1_sb = const.tile([128, NHID, NCC, 128], BF16, tag="w1")
    w2_sb = const.tile([128, NHID, C], BF16, tag="w2")
    for hg in range(4):
        hsl = slice(hg * 8, (hg + 1) * 8)
        nc.sync.dma_start(out=w1_sb[:, hsl, :, :], in_=w1[:, hsl, :, :])
    for hg in range(4):
        hsl = slice(hg * 8, (hg + 1) * 8)
        nc.sync.dma_start(out=w2_sb[:, hsl, :], in_=w2[:, hsl, :])

    hp = ctx.enter_context(tc.tile_pool(name="hp", bufs=5))
    hTp = ctx.enter_context(tc.tile_pool(name="hTp", bufs=1))
    stat = ctx.enter_context(tc.tile_pool(name="stat", bufs=2))
    ftp = ctx.enter_context(tc.tile_pool(name="ftp", bufs=NHID))
    tmp = ctx.enter_context(tc.tile_pool(name="tmp", bufs=2))
    osp = ctx.enter_context(tc.tile_pool(name="osp", bufs=2))

    h2T = hTp.tile([128, NCC, RPC], BF16, tag="h2T")

    # ---- LN2 + transpose ----
    with tc.tile_pool(name="pst", bufs=2, space="PSUM") as PST:
        mvs = stat.tile([128, NRB, 2], F32, tag="mvs")
        rstd = stat.tile([128, NRB], F32, tag="rstd")
        lnv = stat.tile([128, NRB], F32, tag="lnv")
        for r in range(NRB):
            st = stat.tile([128, 2, 6], F32, tag="bn", name=f"bn_{r}")
            for k in range(2):
                nc.vector.bn_stats(out=st[:, k, :],
                                   in_=x2ts[r][:, k * 512:(k + 1) * 512])
            nc.vector.bn_aggr(out=mvs[:, r, :], in_=st)
        nc.scalar.activation(out=lnv, in_=mvs[:, :, 1], func=AF.Ln,
                             bias=eps_t)
        nc.scalar.activation(out=rstd, in_=lnv, func=AF.Exp, scale=-0.5)
        hts = []
        for r in range(NRB):
            ht = hp.tile([128, C], BF16, tag="h", name=f"h_{r}")
            nc.vector.tensor_scalar(
                out=ht, in0=x2ts[r], scalar1=mvs[:, r, 0:1],
                scalar2=rstd[:, r:r + 1], op0=ALU.subtract, op1=ALU.mult)
            if general_ln:
                nc.vector.tensor_mul(out=ht, in0=ht, in1=lnw_bc)
                nc.vector.tensor_add(out=ht, in0=ht, in1=lnb_bc)
            hts.append(ht)
        for cc in range(NCC):
            pt = PST.tile([128, RPC], BF16, tag="tr", name=f"pt_{cc}")
            for r in range(NRB):
                nc.tensor.transpose(
                    pt[:, r * 128:(r + 1) * 128],
                    hts[r][:, cc * 128:(cc + 1) * 128], ident)
            nc.vector.tensor_copy(out=h2T[:, cc, :], in_=pt)

    # ---- W1 + PReLU ----
    f_tiles = []
    with tc.tile_pool(name="psf", bufs=2, space="PSUM") as PSF:
        for h in range(NHID):
            pf = PSF.tile([128, RPC], F32, tag="f", name=f"pf_{h}")
            for cc in range(NCC):
                nc.tensor.matmul(pf, w1_sb[:, h, cc, :], h2T[:, cc, :],
                                 start=(cc == 0), stop=(cc == NCC - 1))
            if b1_sb is not None:
                nc.vector.tensor_scalar_add(out=pf, in0=pf,
                                            scalar1=b1_sb[:, h:h + 1])
            t1 = tmp.tile([128, RPC], F32, tag="t1", name=f"t1_{h}")
            nc.vector.tensor_scalar(
                out=t1, in0=pf, scalar1=0.0, scalar2=alpha - 1.0,
                op0=ALU.min, op1=ALU.mult)
            ft = ftp.tile([128, RPC], BF16, tag="ft", name=f"ft_{h}")
            nc.vector.tensor_add(out=ft, in0=pf, in1=t1)
            f_tiles.append(ft)

    # ---- W2 + residual ----
    with tc.tile_pool(name="pso", bufs=2, space="PSUM") as PSO:
        for tb in range(NRB):
            po = PSO.tile([128, C], F32, tag="o", name=f"po_{tb}")
            for h in range(NHID):
                for co in range(2):
                    nc.tensor.matmul(
                        po[:, co * 512:(co + 1) * 512],
                        f_tiles[h][:, tb * 128:(tb + 1) * 128],
                        w2_sb[:, h, co * 512:(co + 1) * 512],
                        start=(h == 0), stop=(h == NHID - 1))
            o_sb = osp.tile([128, C], F32, tag="osb", name=f"osb_{tb}")
            nc.vector.tensor_add(out=o_sb, in0=po, in1=x2ts[tb])
            nc.sync.dma_start(out=out[tb * 128:(tb + 1) * 128, :], in_=o_sb)


def _build_ffn(general_ln: bool, has_b1: bool, alpha: float):
    nc = bacc.Bacc("TRN2", target_bir_lowering=False, debug=False)
    x2 = nc.dram_tensor("x2", [RPC, C], F32, kind="ExternalInput").ap()
    w1 = nc.dram_tensor("w1", [128, NHID, NCC, 128], BF16,
                        kind="ExternalInput").ap()
    w2 = nc.dram_tensor("w2", [128, NHID, C], BF16,
                        kind="ExternalInput").ap()
    b1 = lnw = lnb = None
    if has_b1:
        b1 = nc.dram_tensor("b1", [4 * C], F32, kind="ExternalInput").ap()
    if general_ln:
        lnw = nc.dram_tensor("lnw", [C], F32, kind="ExternalInput").ap()
        lnb = nc.dram_tensor("lnb", [C], F32, kind="ExternalInput").ap()
    out = nc.dram_tensor("out", [RPC, C], F32, kind="ExternalOutput").ap()
    with tile.TileContext(nc) as tc:
        with ExitStack() as ctx:
            _ffn_body(ctx, tc, x2, w1, w2, b1, lnw, lnb, alpha, out)
    nc.compile()
    return nc


# --------------------------------------------------------------------------
# host orchestration
# --------------------------------------------------------------------------

_NC_CACHE = {}

# bench-only instrumentation: when KBENCH_TRACE is set, launches run with
# trace=True and per-launch device exec_time_ns is appended here.
_TRACE = bool(os.environ.get("KBENCH_TRACE"))
EXEC_NS = []
TRACE_PATHS = []


def _run_spmd(nc, in_maps):
    res = run_bass_kernel_spmd(nc, in_maps, list(range(NCORES)),
                               trace=_TRACE,
                               trace_cores=list(range(NCORES)) if _TRACE
                               else None)
    if _TRACE:
        EXEC_NS.append(res.exec_time_ns)
        if res.instructions_and_trace is not None:
            TRACE_PATHS.append(res.instructions_and_trace[1])
    return res


def _bf16(a):
    import ml_dtypes
    return np.ascontiguousarray(np.asarray(a, np.float32)
                                .astype(ml_dtypes.bfloat16))


def _get_attn_nc(general_ln):
    key = ("attn", general_ln)
    if key not in _NC_CACHE:
        _NC_CACHE[key] = _build_attn(general_ln)
    return _NC_CACHE[key]


def _get_ffn_nc(general_ln, has_b1, alpha):
    key = ("ffn", general_ln, has_b1, float(alpha))
    if key not in _NC_CACHE:
        _NC_CACHE[key] = _build_ffn(general_ln, has_b1, float(alpha))
    return _NC_CACHE[key]


def _attn_weights(Wq, Wk, Wv, Wo):
    """Per-core weight arrays in the device layouts."""
    per_core = []
    for c in range(NCORES):
        hh = c % 2
        h0 = HPC * hh
        # [C, 512] -> [128, NCC, 512]
        def wlay(Wx):
            catw = np.concatenate([Wx[h] for h in range(h0, h0 + HPC)],
                                  axis=1)  # [C, 512]
            return _bf16(catw.reshape(NCC, 128, 512).transpose(1, 0, 2))
        wo = _bf16(Wo[hh * 512:(hh + 1) * 512].reshape(4, 128, C)
                   .transpose(1, 0, 2))
        per_core.append((wlay(Wq), wlay(Wk), wlay(Wv), wo))
    return per_core


def run_attn(x_flat, Wq, Wk, Wv, Wo, ln1_w, ln1_b):
    """Returns list of per-core partial projections [T, C] f32."""
    trivial = bool(np.all(ln1_w == 1.0) and np.all(ln1_b == 0.0))
    nc = _get_attn_nc(not trivial)
    wts = _attn_weights(Wq, Wk, Wv, Wo)
    in_maps = []
    for c in range(NCORES):
        b = c // 2
        wq, wk, wv, wo = wts[c]
        import ml_dtypes
        m = {"x": np.ascontiguousarray(x_flat[b * T:(b + 1) * T]),
             "wq": wq, "wk": wk, "wv": wv, "wo": wo,
             "ones": np.ones((1, 64), ml_dtypes.bfloat16)}
        if not trivial:
            m["lnw"] = np.asarray(ln1_w, np.float32)
            m["lnb"] = np.asarray(ln1_b, np.float32)
        in_maps.append(m)
    res = _run_spmd(nc, in_maps)
    return [res.results[c]["pout"] for c in range(NCORES)]


def run_ffn(x2_flat, W1, b1, W2, ln2_w, ln2_b, alpha):
    trivial = bool(np.all(ln2_w == 1.0) and np.all(ln2_b == 0.0))
    has_b1 = bool(np.any(b1 != 0.0))
    nc = _get_ffn_nc(not trivial, has_b1, alpha)
    w1l = _bf16(np.asarray(W1, np.float32)
                .reshape(NCC, 128, NHID, 128).transpose(1, 2, 0, 3))
    w2l = _bf16(np.asarray(W2, np.float32)
                .reshape(NHID, 128, C).transpose(1, 0, 2))
    in_maps = []
    for c in range(NCORES):
        m = {"x2": np.ascontiguousarray(x2_flat[RPC * c:RPC * (c + 1)]),
             "w1": w1l, "w2": w2l}
        if has_b1:
            m["b1"] = np.asarray(b1, np.float32)
        if not trivial:
            m["lnw"] = np.asarray(ln2_w, np.float32)
            m["lnb"] = np.asarray(ln2_b, np.float32)
        in_maps.append(m)
    res = _run_spmd(nc, in_maps)
    return np.concatenate(
        [res.results[c]["out"] for c in range(NCORES)], axis=0)


def kernel(x, ln1_w, ln1_b, Wk, Wq, Wv, Wo, bo, ln2_w, ln2_b, W1, b1,
           prelu_a, W2, b2):
    x = np.asarray(x, np.float32)
    x_flat = np.ascontiguousarray(x.reshape(B * T, C))
    alpha = float(np.asarray(prelu_a))

    parts = run_attn(x_flat, np.asarray(Wq, np.float32),
                     np.asarray(Wk, np.float32),
                     np.asarray(Wv, np.float32),
                     np.asarray(Wo, np.float32),
                     np.asarray(ln1_w, np.float32),
                     np.asarray(ln1_b, np.float32))
    # host reduction: x2 = x + partial_even + partial_odd (+ bo)
    x2 = np.empty_like(x_flat)
    for b in range(B):
        x2[b * T:(b + 1) * T] = (x_flat[b * T:(b + 1) * T]
                                 + parts[2 * b].astype(np.float32)
                                 + parts[2 * b + 1].astype(np.float32))
    bo = np.asarray(bo, np.float32)
    if np.any(bo != 0.0):
        x2 += bo
    out = run_ffn(x2, W1, np.asarray(b1, np.float32), W2,
                  np.asarray(ln2_w, np.float32),
                  np.asarray(ln2_b, np.float32), alpha)
    b2 = np.asarray(b2, np.float32)
    if np.any(b2 != 0.0):
        out = out + b2
    return out.reshape(B, T, C).astype(np.float32)


# revision 38
# speedup vs baseline: 1.0654x; 1.0654x over previous
"""Trainium2 Bass kernel for a dense pre-LN transformer block.

B=4, T=1024, C=1024, H=16 heads (head_size 64).

Distribution over 8 NeuronCores, two SPMD launches with a free host-side
reduction between them:

  Launch A (attention): core c works on batch b=c//2 and head-half
  hh=c%2 (8 heads). It computes LN1 for its batch only, projects
  q/k/v for its heads, runs causal softmax(k@q^T)-attention in the
  transposed-scores layout, and multiplies by its slice of Wo rows,
  producing a PARTIAL projection [T, C] (f32) for its batch.

  Host: x2[b] = x[b] + part[2b] + part[2b+1] (+bo).

  Launch B (FFN): core c runs LN2 + W1/PReLU/W2 + residual on rows
  [512c, 512(c+1)) of x2.

Matmul dtype strategy: the PE cost depends only on the MOVING operand
dtype and its free size (1 cycle/row for bf16 at any N, f32r at N>=256).
Activations that move (hT, wo, w2 stream, e) stay f32r/bf16 chosen for
SBUF fit; weights that sit stationary are bf16 (0.4% quantization).
Accumulation is always f32 in PSUM.
"""

import os
from contextlib import ExitStack

import numpy as np

import concourse.bass as bass
import concourse.tile as tile
from concourse import bacc, mybir
from concourse.bass_utils import run_bass_kernel_spmd
from concourse.masks import make_identity

F32 = mybir.dt.float32
F32R = mybir.dt.float32r
BF16 = mybir.dt.bfloat16
FP8 = mybir.dt.float8e4
AF = mybir.ActivationFunctionType
ALU = mybir.AluOpType

B, T, C, H, HS = 4, 1024, 1024, 16, 64
NCORES = 8
EPS = 1e-5
SCALE = float(C) ** -0.5  # folded into the softmax exp
NEG = -1e30

NTB = T // 128   # 8 token blocks per batch
NCC = C // 128   # 8 channel chunks
HPC = H // 2     # 8 heads per core
RPC = (B * T) // NCORES  # 512 rows per core in launch B
NRB = RPC // 128         # 4 row blocks
NHID = 4 * C // 128      # 32 hidden chunks


def _bank_chunks(lo, hi):
    """Split [lo, hi) at 512-column PSUM bank boundaries."""
    out = []
    o = lo
    while o < hi:
        n = min(512 - (o % 512), hi - o)
        out.append((o, o + n))
        o += n
    return out


# --------------------------------------------------------------------------
# kernel A: attention, one batch + 8 heads per core
# --------------------------------------------------------------------------

def _attn_body(ctx, tc, x, wq, wk, wv, wo, lnw, lnb, ones_dram, pout):
    nc = tc.nc
    general_ln = lnw is not None

    const = ctx.enter_context(tc.tile_pool(name="const", bufs=1))
    hTp = ctx.enter_context(tc.tile_pool(name="hTp", bufs=1))
    qTp = ctx.enter_context(tc.tile_pool(name="qTp", bufs=4))
    kTp = ctx.enter_context(tc.tile_pool(name="kTp", bufs=4))
    v2p = ctx.enter_context(tc.tile_pool(name="v2p", bufs=1))
    stat = ctx.enter_context(tc.tile_pool(name="stat", bufs=2))
    ep = ctx.enter_context(tc.tile_pool(name="ep", bufs=2))
    avp = ctx.enter_context(tc.tile_pool(name="avp", bufs=4))
    ctp = ctx.enter_context(tc.tile_pool(name="ctp", bufs=4))
    osp = ctx.enter_context(tc.tile_pool(name="osp", bufs=2))

    xp_cm = tc.tile_pool(name="xp", bufs=8)
    hp_cm = tc.tile_pool(name="hp", bufs=3)
    xp = xp_cm.__enter__()
    hp = hp_cm.__enter__()
    # x tiles first: these DMAs gate the LN1 critical path
    xts = []
    for i in range(NTB):
        xt = xp.tile([128, C], BF16, tag="x", name=f"x_{i}")
        nc.sync.dma_start(out=xt, in_=x[i * 128:(i + 1) * 128, :])
        xts.append(xt)

    scratch = const.tile([128, 128], F32)
    make_identity(nc, scratch)
    ident = const.tile([128, 128], BF16)
    nc.vector.tensor_copy(out=ident, in_=scratch)
    # transposed causal mask for diagonal blocks: keep s<=t (cols>=rows)
    trilT = const.tile([128, 128], F32)
    nc.gpsimd.memset(trilT, 0.0)
    nc.gpsimd.affine_select(
        out=trilT, in_=trilT, compare_op=ALU.is_ge, fill=NEG, base=0,
        pattern=[[1, 128]], channel_multiplier=-1)
    eps_t = const.tile([128, 1], F32)
    nc.vector.memset(eps_t, EPS)
    ones64 = const.tile([1, 64], BF16)
    nc.sync.dma_start(out=ones64, in_=ones_dram)
    if general_ln:
        lnw_bc = const.tile([128, C], F32, tag="lnw")
        lnb_bc = const.tile([128, C], F32, tag="lnb")
        nc.sync.dma_start(
            out=lnw_bc,
            in_=bass.AP(tensor=lnw.tensor, offset=lnw.offset,
                        ap=[[0, 128]] + list(lnw.ap)))
        nc.sync.dma_start(
            out=lnb_bc,
            in_=bass.AP(tensor=lnb.tensor, offset=lnb.offset,
                        ap=[[0, 128]] + list(lnb.ap)))

    # weights resident in SBUF (after x: LN1 must not wait behind these)
    wq_sb = const.tile([128, NCC, 512], BF16, tag="wq")
    wk_sb = const.tile([128, NCC, 512], BF16, tag="wk")
    wv_sb = const.tile([128, NCC, 512], BF16, tag="wv")
    wo_sb = const.tile([128, 4, C], BF16, tag="wo")
    nc.sync.dma_start(out=wq_sb, in_=wq)
    nc.sync.dma_start(out=wk_sb, in_=wk)
    nc.sync.dma_start(out=wv_sb, in_=wv)
    nc.sync.dma_start(out=wo_sb, in_=wo)

    hT = hTp.tile([128, NCC, T], BF16, tag="hT")

    # ---- Phase A: LN1 (own batch only) + transpose, 2 groups of 4 ----
    with tc.tile_pool(name="pst", bufs=2, space="PSUM") as PST:
        for grp in range(2):
            mvs = stat.tile([128, 4, 2], F32, tag="mvs", name=f"mvs_{grp}")
            rstd = stat.tile([128, 4], F32, tag="rstd", name=f"rstd_{grp}")
            lnv = stat.tile([128, 4], F32, tag="lnv", name=f"lnv_{grp}")
            for j in range(4):
                i = grp * 4 + j
                st = stat.tile([128, 2, 6], F32, tag="bn", name=f"bn_{i}")
                for k in range(2):
                    nc.vector.bn_stats(out=st[:, k, :],
                                       in_=xts[i][:, k * 512:(k + 1) * 512])
                nc.vector.bn_aggr(out=mvs[:, j, :], in_=st)
            nc.scalar.activation(out=lnv, in_=mvs[:, :, 1], func=AF.Ln,
                                 bias=eps_t)
            nc.scalar.activation(out=rstd, in_=lnv, func=AF.Exp, scale=-0.5)
            for j in range(4):
                i = grp * 4 + j
                ht = hp.tile([128, C], BF16, tag="h", name=f"h_{i}")
                nc.vector.tensor_scalar(
                    out=ht, in0=xts[i], scalar1=mvs[:, j, 0:1],
                    scalar2=rstd[:, j:j + 1], op0=ALU.subtract, op1=ALU.mult)
                if general_ln:
                    nc.vector.tensor_mul(out=ht, in0=ht, in1=lnw_bc)
                    nc.vector.tensor_add(out=ht, in0=ht, in1=lnb_bc)
                for g in range(2):
                    pt = PST.tile([128, 512], BF16, tag="tr",
                                  name=f"pt_{i}_{g}")
                    for c in range(4):
                        cc = g * 4 + c
                        nc.tensor.transpose(
                            pt[:, c * 128:(c + 1) * 128],
                            ht[:, cc * 128:(cc + 1) * 128], ident)
                    for c in range(4):
                        cc = g * 4 + c
                        nc.vector.tensor_copy(
                            out=hT[:, cc, i * 128:(i + 1) * 128],
                            in_=pt[:, c * 128:(c + 1) * 128])
    hp_cm.__exit__(None, None, None)
    xp_cm.__exit__(None, None, None)
    dnp = ctx.enter_context(tc.tile_pool(name="dnp", bufs=8))
    lnp = ctx.enter_context(tc.tile_pool(name="lnp", bufs=8))
    recp = ctx.enter_context(tc.tile_pool(name="recp", bufs=8))

    # ---- Phase B+C+D interleaved: qkv, scores+exp, av ----
    qT = [qTp.tile([128, T], BF16, tag="qT", name=f"qT_{m}")
          for m in range(4)]
    kT = [kTp.tile([128, T], BF16, tag="kT", name=f"kT_{m}")
          for m in range(4)]
    # v2[tok, sc, h, 0:64] = v; col 64 = ones (softmax denominator)
    v2 = v2p.tile([128, NTB, HPC, 66], BF16, tag="v2")
    nc.vector.memset(v2[:, :, :, 64:65], 1.0)
    nc.vector.memset(v2[:, :, :, 65:66], 0.0)
    cat = [ctp.tile([128, T], BF16, tag="cat", name=f"cat_{m}")
           for m in range(4)]
    # per-head softmax denominator rows and their reciprocals
    dens = [dnp.tile([1, T], BF16, tag="den", name=f"den_{h}")
            for h in range(HPC)]
    lnds = [lnp.tile([1, T], F32, tag="lnd", name=f"lnd_{h}")
            for h in range(HPC)]
    recs = [recp.tile([1, T], BF16, tag="rec", name=f"rec_{h}")
            for h in range(HPC)]
    pav_sbs = [avp.tile([128, T], F32R, tag="avsb", name=f"avsb_{m}")
               for m in range(4)]

    # PSUM budget: PSQ 1x[128,512]=1 bank, PSS 3x[128,<=512]=3,
    # PSA 2x[66,1024]=4  -> 8 banks total.
    with tc.tile_pool(name="psq", bufs=1, space="PSUM") as PSQ, \
         tc.tile_pool(name="pss", bufs=3, space="PSUM") as PSS, \
         tc.tile_pool(name="psa", bufs=2, space="PSUM") as PSA:

        def qkv_m(m):
            for th in range(2):
                tsl = slice(th * 512, (th + 1) * 512)
                pq = PSQ.tile([128, 512], F32, tag="mm", name=f"pq_{m}_{th}")
                for cc in range(NCC):
                    nc.tensor.matmul(pq, wq_sb[:, cc, m * 128:(m + 1) * 128],
                                     hT[:, cc, tsl],
                                     start=(cc == 0), stop=(cc == NCC - 1))
                nc.vector.tensor_copy(out=qT[m][:, tsl], in_=pq)
                pk = PSQ.tile([128, 512], F32, tag="mm", name=f"pk_{m}_{th}")
                for cc in range(NCC):
                    nc.tensor.matmul(pk, wk_sb[:, cc, m * 128:(m + 1) * 128],
                                     hT[:, cc, tsl],
                                     start=(cc == 0), stop=(cc == NCC - 1))
                nc.vector.tensor_copy(out=kT[m][:, tsl], in_=pk)

        def v_tb(tb):
            pv = PSQ.tile([128, 512], F32, tag="mm", name=f"pv_{tb}")
            for cc in range(NCC):
                nc.tensor.matmul(pv, hT[:, cc, tb * 128:(tb + 1) * 128],
                                 wv_sb[:, cc, :],
                                 start=(cc == 0), stop=(cc == NCC - 1))
            nc.vector.tensor_copy(
                out=v2[:, tb, :, 0:64],
                in_=bass.AP(tensor=pv.tensor, offset=pv.offset,
                            ap=list(pv.ap[:1]) + [[64, HPC], [1, 64]]))

        def scores_h(h):
            """scoresT + exp for head h; returns e tiles per sc."""
            m, ho = h // 2, (h % 2) * 64
            qh = qT[m][ho:ho + 64, :]
            kh = kT[m][ho:ho + 64, :]
            es = []
            for sc in range(NTB):
                W = T - sc * 128
                e = ep.tile([128, W], BF16, tag=f"e{sc}", name=f"e_{h}_{sc}")
                for (o0, o1) in _bank_chunks(0, W):
                    ps = PSS.tile([128, o1 - o0], F32, tag="sc",
                                  name=f"ps_{h}_{sc}_{o0}")
                    nc.tensor.matmul(
                        ps,
                        qh[:, sc * 128:(sc + 1) * 128],
                        kh[:, sc * 128 + o0:sc * 128 + o1],
                        start=True, stop=True)
                    if o0 == 0:
                        nc.vector.tensor_add(out=ps[:, 0:128],
                                             in0=ps[:, 0:128], in1=trilT)
                    nc.scalar.activation(out=e[:, o0:o1], in_=ps,
                                         func=AF.Exp, scale=SCALE)
                es.append(e)
            return es

        def av_h(h, es):
            """av for head h into pav_sb half + stash denominator row."""
            pav = PSA.tile([66, T], F32, tag="av", name=f"pav_{h}")
            for sc in range(NTB):
                lo = sc * 128
                for (o0, o1) in _bank_chunks(lo, T):
                    nc.tensor.matmul(
                        pav[:, o0:o1], v2[:, sc, h, :],
                        es[sc][:, o0 - lo:o1 - lo],
                        start=(sc == 0), stop=(sc == NTB - 1),
                        skip_group_check=True)
            ho = (h % 2) * 64
            nc.vector.tensor_copy(out=pav_sbs[h // 2][ho:ho + 64, :],
                                  in_=pav[0:64, :])
            nc.scalar.copy(out=dens[h], in_=pav[64:65, :])

        es0 = qkv_m(0) or scores_h(0)
        qkv_m(1)
        for tb in range(NTB):
            v_tb(tb)
        es1 = scores_h(1)
        qkv_m(2)
        es2 = scores_h(2)
        qkv_m(3)

        es = {0: es0, 1: es1, 2: es2}
        for h in range(HPC):
            if h + 3 < HPC:
                es[h + 3] = scores_h(h + 3)
            av_h(h, es.pop(h))

    # ---- Phase E: 1/den via exp(-ln(den)), normalize, Wo projection ----
    with tc.tile_pool(name="psb", bufs=2, space="PSUM") as PSB, \
         tc.tile_pool(name="psp", bufs=2, space="PSUM") as PSP:
        with nc.allow_low_precision(reason="softmax denominator recip"):
            for grp in range(2):
                for i in range(4):
                    nc.scalar.activation(out=lnds[grp * 4 + i],
                                         in_=dens[grp * 4 + i], func=AF.Ln)
                for i in range(4):
                    nc.scalar.activation(out=recs[grp * 4 + i],
                                         in_=lnds[grp * 4 + i],
                                         func=AF.Exp, scale=-1.0)
        for m in range(4):
            # partition-broadcast each head's 1/den row via K=1 PE matmul
            prec = PSB.tile([128, T], F32, tag="prec", name=f"prec_{m}")
            for hh in range(2):
                for co in range(2):
                    nc.tensor.matmul(
                        prec[hh * 64:(hh + 1) * 64, co * 512:(co + 1) * 512],
                        ones64,
                        recs[2 * m + hh][:, co * 512:(co + 1) * 512],
                        start=True, stop=True)
            nc.vector.tensor_mul(out=cat[m], in0=pav_sbs[m], in1=prec)

        for tb in range(NTB):
            pp = PSP.tile([128, C], F32, tag="pp", name=f"pp_{tb}")
            for m in range(4):
                for co in range(2):
                    nc.tensor.matmul(
                        pp[:, co * 512:(co + 1) * 512],
                        cat[m][:, tb * 128:(tb + 1) * 128],
                        wo_sb[:, m, co * 512:(co + 1) * 512],
                        start=(m == 0), stop=(m == 3))
            o_sb = osp.tile([128, C], BF16, tag="o", name=f"o_{tb}")
            nc.vector.tensor_copy(out=o_sb, in_=pp)
            nc.sync.dma_start(out=pout[tb * 128:(tb + 1) * 128, :], in_=o_sb)


def _build_attn(general_ln: bool):
    nc = bacc.Bacc("TRN2", target_bir_lowering=False, debug=False)
    x = nc.dram_tensor("x", [T, C], BF16, kind="ExternalInput").ap()
    wq = nc.dram_tensor("wq", [128, NCC, 512], BF16, kind="ExternalInput").ap()
    wk = nc.dram_tensor("wk", [128, NCC, 512], BF16, kind="ExternalInput").ap()
    wv = nc.dram_tensor("wv", [128, NCC, 512], BF16, kind="ExternalInput").ap()
    wo = nc.dram_tensor("wo", [128, 4, C], BF16, kind="ExternalInput").ap()
    lnw = lnb = None
    if general_ln:
        lnw = nc.dram_tensor("lnw", [C], F32, kind="ExternalInput").ap()
        lnb = nc.dram_tensor("lnb", [C], F32, kind="ExternalInput").ap()
    ones_dram = nc.dram_tensor("ones", [1, 64], BF16,
                               kind="ExternalInput").ap()
    pout = nc.dram_tensor("pout", [T, C], BF16, kind="ExternalOutput").ap()
    with tile.TileContext(nc) as tc:
        with ExitStack() as ctx:
            _attn_body(ctx, tc, x, wq, wk, wv, wo, lnw, lnb, ones_dram, pout)
    nc.compile()
    return nc


# --------------------------------------------------------------------------
# kernel B: FFN, 512 rows per core
# --------------------------------------------------------------------------

def _ffn_body(ctx, tc, x2, w1, w2, b1, lnw, lnb, alpha, out):
    nc = tc.nc
    general_ln = lnw is not None

    const = ctx.enter_context(tc.tile_pool(name="const", bufs=1))
    xp = ctx.enter_context(tc.tile_pool(name="xp", bufs=NRB))
    # x2 tiles first: these DMAs gate the LN2 critical path
    x2ts = []
    for r in range(NRB):
        xt = xp.tile([128, C], F32, tag="x", name=f"x_{r}")
        nc.sync.dma_start(out=xt, in_=x2[r * 128:(r + 1) * 128, :])
        x2ts.append(xt)
    scratch = const.tile([128, 128], F32)
    make_identity(nc, scratch)
    ident = const.tile([128, 128], BF16)
    nc.vector.tensor_copy(out=ident, in_=scratch)
    eps_t = const.tile([128, 1], F32)
    nc.vector.memset(eps_t, EPS)
    if general_ln:
        lnw_bc = const.tile([128, C], F32, tag="lnw")
        lnb_bc = const.tile([128, C], F32, tag="lnb")
        nc.sync.dma_start(
            out=lnw_bc,
            in_=bass.AP(tensor=lnw.tensor, offset=lnw.offset,
                        ap=[[0, 128]] + list(lnw.ap)))
        nc.sync.dma_start(
            out=lnb_bc,
            in_=bass.AP(tensor=lnb.tensor, offset=lnb.offset,
                        ap=[[0, 128]] + list(lnb.ap)))
    b1_sb = None
    if b1 is not None:
        b1_sb = const.tile([128, NHID], F32, tag="b1")
        nc.sync.dma_start(out=b1_sb, in_=b1.rearrange("(h p) -> p h", p=128))

    # weights: big resident tiles, streamed in chunks of 8 hidden blocks
    w1_sb = const.tile([128, NHID, NCC, 128], BF16, tag="w1")
    w2_sb = const.tile([128, NHID, C], BF16, tag="w2")
    for hg in range(4):
        hsl = slice(hg * 8, (hg + 1) * 8)
        nc.sync.dma_start(out=w1_sb[:, hsl, :, :], in_=w1[:, hsl, :, :])
    for hg in range(4):
        hsl = slice(hg * 8, (hg + 1) * 8)
        nc.sync.dma_start(out=w2_sb[:, hsl, :], in_=w2[:, hsl, :])

    hp = ctx.enter_context(tc.tile_pool(name="hp", bufs=5))
    hTp = ctx.enter_context(tc.tile_pool(name="hTp", bufs=1))
    stat = ctx.enter_context(tc.tile_pool(name="stat", bufs=2))
    ftp = ctx.enter_context(tc.tile_pool(name="ftp", bufs=1))
    tmp = ctx.enter_context(tc.tile_pool(name="tmp", bufs=2))
    osp = ctx.enter_context(tc.tile_pool(name="osp", bufs=2))

    h2T = hTp.tile([128, NCC, RPC], BF16, tag="h2T")

    # ---- LN2 + transpose ----
    with tc.tile_pool(name="pst", bufs=2, space="PSUM") as PST:
        mvs = stat.tile([128, NRB, 2], F32, tag="mvs")
        rstd = stat.tile([128, NRB], F32, tag="rstd")
        lnv = stat.tile([128, NRB], F32, tag="lnv")
        for r in range(NRB):
            st = stat.tile([128, 2, 6], F32, tag="bn", name=f"bn_{r}")
            for k in range(2):
                nc.vector.bn_stats(out=st[:, k, :],
                                   in_=x2ts[r][:, k * 512:(k + 1) * 512])
            nc.vector.bn_aggr(out=mvs[:, r, :], in_=st)
        nc.scalar.activation(out=lnv, in_=mvs[:, :, 1], func=AF.Ln,
                             bias=eps_t)
        nc.scalar.activation(out=rstd, in_=lnv, func=AF.Exp, scale=-0.5)
        hts = []
        for r in range(NRB):
            ht = hp.tile([128, C], BF16, tag="h", name=f"h_{r}")
            nc.vector.tensor_scalar(
                out=ht, in0=x2ts[r], scalar1=mvs[:, r, 0:1],
                scalar2=rstd[:, r:r + 1], op0=ALU.subtract, op1=ALU.mult)
            if general_ln:
                nc.vector.tensor_mul(out=ht, in0=ht, in1=lnw_bc)
                nc.vector.tensor_add(out=ht, in0=ht, in1=lnb_bc)
            hts.append(ht)
        for cc in range(NCC):
            pt = PST.tile([128, RPC], BF16, tag="tr", name=f"pt_{cc}")
            for r in range(NRB):
                nc.tensor.transpose(
                    pt[:, r * 128:(r + 1) * 128],
                    hts[r][:, cc * 128:(cc + 1) * 128], ident)
            nc.vector.tensor_copy(out=h2T[:, cc, :], in_=pt)

    # ---- W1 + PReLU ----
    fbig = ftp.tile([128, NHID, RPC], BF16, tag="ft")
    with tc.tile_pool(name="psf", bufs=2, space="PSUM") as PSF:
        for h in range(NHID):
            pf = PSF.tile([128, RPC], F32, tag="f", name=f"pf_{h}")
            for cc in range(NCC):
                nc.tensor.matmul(pf, w1_sb[:, h, cc, :], h2T[:, cc, :],
                                 start=(cc == 0), stop=(cc == NCC - 1))
            if b1_sb is not None:
                nc.vector.tensor_scalar_add(out=pf, in0=pf,
                                            scalar1=b1_sb[:, h:h + 1])
            t1 = tmp.tile([128, RPC], F32, tag="t1", name=f"t1_{h}")
            nc.vector.tensor_scalar(
                out=t1, in0=pf, scalar1=0.0, scalar2=alpha - 1.0,
                op0=ALU.min, op1=ALU.mult)
            nc.vector.tensor_add(out=fbig[:, h, :], in0=pf, in1=t1)

    # ---- W2 + residual ----
    with tc.tile_pool(name="pso", bufs=2, space="PSUM") as PSO:
        for tb in range(NRB):
            po = PSO.tile([128, C], F32, tag="o", name=f"po_{tb}")
            for h in range(NHID):
                for co in range(2):
                    nc.tensor.matmul(
                        po[:, co * 512:(co + 1) * 512],
                        fbig[:, h, tb * 128:(tb + 1) * 128],
                        w2_sb[:, h, co * 512:(co + 1) * 512],
                        start=(h == 0), stop=(h == NHID - 1))
            o_sb = osp.tile([128, C], F32, tag="osb", name=f"osb_{tb}")
            nc.vector.tensor_add(out=o_sb, in0=po, in1=x2ts[tb])
            nc.sync.dma_start(out=out[tb * 128:(tb + 1) * 128, :], in_=o_sb)


def _build_ffn(general_ln: bool, has_b1: bool, alpha: float):
    nc = bacc.Bacc("TRN2", target_bir_lowering=False, debug=False)
    x2 = nc.dram_tensor("x2", [RPC, C], F32, kind="ExternalInput").ap()
    w1 = nc.dram_tensor("w1", [128, NHID, NCC, 128], BF16,
                        kind="ExternalInput").ap()
    w2 = nc.dram_tensor("w2", [128, NHID, C], BF16,
                        kind="ExternalInput").ap()
    b1 = lnw = lnb = None
    if has_b1:
        b1 = nc.dram_tensor("b1", [4 * C], F32, kind="ExternalInput").ap()
    if general_ln:
        lnw = nc.dram_tensor("lnw", [C], F32, kind="ExternalInput").ap()
        lnb = nc.dram_tensor("lnb", [C], F32, kind="ExternalInput").ap()
    out = nc.dram_tensor("out", [RPC, C], F32, kind="ExternalOutput").ap()
    with tile.TileContext(nc) as tc:
        with ExitStack() as ctx:
            _ffn_body(ctx, tc, x2, w1, w2, b1, lnw, lnb, alpha, out)
    nc.compile()
    return nc


# --------------------------------------------------------------------------
# host orchestration
# --------------------------------------------------------------------------

_NC_CACHE = {}

# bench-only instrumentation: when KBENCH_TRACE is set, launches run with
# trace=True and per-launch device exec_time_ns is appended here.
_TRACE = bool(os.environ.get("KBENCH_TRACE"))
EXEC_NS = []
TRACE_PATHS = []


def _run_spmd(nc, in_maps):
    res = run_bass_kernel_spmd(nc, in_maps, list(range(NCORES)),
                               trace=_TRACE,
                               trace_cores=list(range(NCORES)) if _TRACE
                               else None)
    if _TRACE:
        EXEC_NS.append(res.exec_time_ns)
        if res.instructions_and_trace is not None:
            TRACE_PATHS.append(res.instructions_and_trace[1])
    return res


def _bf16(a):
    import ml_dtypes
    return np.ascontiguousarray(np.asarray(a, np.float32)
                                .astype(ml_dtypes.bfloat16))


def _get_attn_nc(general_ln):
    key = ("attn", general_ln)
    if key not in _NC_CACHE:
        _NC_CACHE[key] = _build_attn(general_ln)
    return _NC_CACHE[key]


def _get_ffn_nc(general_ln, has_b1, alpha):
    key = ("ffn", general_ln, has_b1, float(alpha))
    if key not in _NC_CACHE:
        _NC_CACHE[key] = _build_ffn(general_ln, has_b1, float(alpha))
    return _NC_CACHE[key]


def _attn_weights(Wq, Wk, Wv, Wo):
    """Per-core weight arrays in the device layouts."""
    per_core = []
    for c in range(NCORES):
        hh = c % 2
        h0 = HPC * hh
        # [C, 512] -> [128, NCC, 512]
        def wlay(Wx):
            catw = np.concatenate([Wx[h] for h in range(h0, h0 + HPC)],
                                  axis=1)  # [C, 512]
            return _bf16(catw.reshape(NCC, 128, 512).transpose(1, 0, 2))
        wo = _bf16(Wo[hh * 512:(hh + 1) * 512].reshape(4, 128, C)
                   .transpose(1, 0, 2))
        per_core.append((wlay(Wq), wlay(Wk), wlay(Wv), wo))
    return per_core


def run_attn(x_flat, Wq, Wk, Wv, Wo, ln1_w, ln1_b):
    """Returns list of per-core partial projections [T, C] f32."""
    trivial = bool(np.all(ln1_w == 1.0) and np.all(ln1_b == 0.0))
    nc = _get_attn_nc(not trivial)
    wts = _attn_weights(Wq, Wk, Wv, Wo)
    in_maps = []
    for c in range(NCORES):
        b = c // 2
        wq, wk, wv, wo = wts[c]
        import ml_dtypes
        m = {"x": _bf16(x_flat[b * T:(b + 1) * T]),
             "wq": wq, "wk": wk, "wv": wv, "wo": wo,
             "ones": np.ones((1, 64), ml_dtypes.bfloat16)}
        if not trivial:
            m["lnw"] = np.asarray(ln1_w, np.float32)
            m["lnb"] = np.asarray(ln1_b, np.float32)
        in_maps.append(m)
    res = _run_spmd(nc, in_maps)
    return [res.results[c]["pout"] for c in range(NCORES)]


def run_ffn(x2_flat, W1, b1, W2, ln2_w, ln2_b, alpha):
    trivial = bool(np.all(ln2_w == 1.0) and np.all(ln2_b == 0.0))
    has_b1 = bool(np.any(b1 != 0.0))
    nc = _get_ffn_nc(not trivial, has_b1, alpha)
    w1l = _bf16(np.asarray(W1, np.float32)
                .reshape(NCC, 128, NHID, 128).transpose(1, 2, 0, 3))
    w2l = _bf16(np.asarray(W2, np.float32)
                .reshape(NHID, 128, C).transpose(1, 0, 2))
    in_maps = []
    for c in range(NCORES):
        m = {"x2": np.ascontiguousarray(x2_flat[RPC * c:RPC * (c + 1)]),
             "w1": w1l, "w2": w2l}
        if has_b1:
            m["b1"] = np.asarray(b1, np.float32)
        if not trivial:
            m["lnw"] = np.asarray(ln2_w, np.float32)
            m["lnb"] = np.asarray(ln2_b, np.float32)
        in_maps.append(m)
    res = _run_spmd(nc, in_maps)
    return np.concatenate(
        [res.results[c]["out"] for c in range(NCORES)], axis=0)


def kernel(x, ln1_w, ln1_b, Wk, Wq, Wv, Wo, bo, ln2_w, ln2_b, W1, b1,
           prelu_a, W2, b2):
    x = np.asarray(x, np.float32)
    x_flat = np.ascontiguousarray(x.reshape(B * T, C))
    alpha = float(np.asarray(prelu_a))

    parts = run_attn(x_flat, np.asarray(Wq, np.float32),
                     np.asarray(Wk, np.float32),
                     np.asarray(Wv, np.float32),
                     np.asarray(Wo, np.float32),
                     np.asarray(ln1_w, np.float32),
                     np.asarray(ln1_b, np.float32))
    # host reduction: x2 = x + partial_even + partial_odd (+ bo)
    x2 = np.empty_like(x_flat)
    for b in range(B):
        x2[b * T:(b + 1) * T] = (x_flat[b * T:(b + 1) * T]
                                 + parts[2 * b].astype(np.float32)
                                 + parts[2 * b + 1].astype(np.float32))
    bo = np.asarray(bo, np.float32)
    if np.any(bo != 0.0):
        x2 += bo
    out = run_ffn(x2, W1, np.asarray(b1, np.float32), W2,
                  np.asarray(ln2_w, np.float32),
                  np.asarray(ln2_b, np.float32), alpha)
    b2 = np.asarray(b2, np.float32)
    if np.any(b2 != 0.0):
        out = out + b2
    return out.reshape(B, T, C).astype(np.float32)


# revision 39
# speedup vs baseline: 1.0722x; 1.0063x over previous
"""Trainium2 Bass kernel for a dense pre-LN transformer block.

B=4, T=1024, C=1024, H=16 heads (head_size 64).

Distribution over 8 NeuronCores, two SPMD launches with a free host-side
reduction between them:

  Launch A (attention): core c works on batch b=c//2 and head-half
  hh=c%2 (8 heads). It computes LN1 for its batch only, projects
  q/k/v for its heads, runs causal softmax(k@q^T)-attention in the
  transposed-scores layout, and multiplies by its slice of Wo rows,
  producing a PARTIAL projection [T, C] (f32) for its batch.

  Host: x2[b] = x[b] + part[2b] + part[2b+1] (+bo).

  Launch B (FFN): core c runs LN2 + W1/PReLU/W2 + residual on rows
  [512c, 512(c+1)) of x2.

Matmul dtype strategy: the PE cost depends only on the MOVING operand
dtype and its free size (1 cycle/row for bf16 at any N, f32r at N>=256).
Activations that move (hT, wo, w2 stream, e) stay f32r/bf16 chosen for
SBUF fit; weights that sit stationary are bf16 (0.4% quantization).
Accumulation is always f32 in PSUM.
"""

import os
from contextlib import ExitStack

import numpy as np

import concourse.bass as bass
import concourse.tile as tile
from concourse import bacc, mybir
from concourse.bass_utils import run_bass_kernel_spmd
from concourse.masks import make_identity

F32 = mybir.dt.float32
F32R = mybir.dt.float32r
BF16 = mybir.dt.bfloat16
FP8 = mybir.dt.float8e4
AF = mybir.ActivationFunctionType
ALU = mybir.AluOpType

B, T, C, H, HS = 4, 1024, 1024, 16, 64
NCORES = 8
EPS = 1e-5
SCALE = float(C) ** -0.5  # folded into the softmax exp
NEG = -1e30

NTB = T // 128   # 8 token blocks per batch
NCC = C // 128   # 8 channel chunks
HPC = H // 2     # 8 heads per core
RPC = (B * T) // NCORES  # 512 rows per core in launch B
NRB = RPC // 128         # 4 row blocks
NHID = 4 * C // 128      # 32 hidden chunks


def _bank_chunks(lo, hi):
    """Split [lo, hi) at 512-column PSUM bank boundaries."""
    out = []
    o = lo
    while o < hi:
        n = min(512 - (o % 512), hi - o)
        out.append((o, o + n))
        o += n
    return out


# --------------------------------------------------------------------------
# kernel A: attention, one batch + 8 heads per core
# --------------------------------------------------------------------------

def _attn_body(ctx, tc, x, wq, wk, wv, wo, lnw, lnb, ones_dram, pout):
    nc = tc.nc
    general_ln = lnw is not None

    const = ctx.enter_context(tc.tile_pool(name="const", bufs=1))
    hTp = ctx.enter_context(tc.tile_pool(name="hTp", bufs=1))
    qTp = ctx.enter_context(tc.tile_pool(name="qTp", bufs=4))
    kTp = ctx.enter_context(tc.tile_pool(name="kTp", bufs=4))
    v2p = ctx.enter_context(tc.tile_pool(name="v2p", bufs=1))
    stat = ctx.enter_context(tc.tile_pool(name="stat", bufs=2))
    ep = ctx.enter_context(tc.tile_pool(name="ep", bufs=3))
    avp = ctx.enter_context(tc.tile_pool(name="avp", bufs=4))
    ctp = ctx.enter_context(tc.tile_pool(name="ctp", bufs=4))
    osp = ctx.enter_context(tc.tile_pool(name="osp", bufs=2))

    xp_cm = tc.tile_pool(name="xp", bufs=8)
    hp_cm = tc.tile_pool(name="hp", bufs=3)
    xp = xp_cm.__enter__()
    hp = hp_cm.__enter__()
    # x tiles first: these DMAs gate the LN1 critical path
    xts = []
    for i in range(NTB):
        xt = xp.tile([128, C], BF16, tag="x", name=f"x_{i}")
        nc.sync.dma_start(out=xt, in_=x[i * 128:(i + 1) * 128, :])
        xts.append(xt)

    scratch = const.tile([128, 128], F32)
    make_identity(nc, scratch)
    ident = const.tile([128, 128], BF16)
    nc.vector.tensor_copy(out=ident, in_=scratch)
    # transposed causal mask for diagonal blocks: keep s<=t (cols>=rows)
    trilT = const.tile([128, 128], F32)
    nc.gpsimd.memset(trilT, 0.0)
    nc.gpsimd.affine_select(
        out=trilT, in_=trilT, compare_op=ALU.is_ge, fill=NEG, base=0,
        pattern=[[1, 128]], channel_multiplier=-1)
    eps_t = const.tile([128, 1], F32)
    nc.vector.memset(eps_t, EPS)
    ones64 = const.tile([1, 64], BF16)
    nc.sync.dma_start(out=ones64, in_=ones_dram)
    if general_ln:
        lnw_bc = const.tile([128, C], F32, tag="lnw")
        lnb_bc = const.tile([128, C], F32, tag="lnb")
        nc.sync.dma_start(
            out=lnw_bc,
            in_=bass.AP(tensor=lnw.tensor, offset=lnw.offset,
                        ap=[[0, 128]] + list(lnw.ap)))
        nc.sync.dma_start(
            out=lnb_bc,
            in_=bass.AP(tensor=lnb.tensor, offset=lnb.offset,
                        ap=[[0, 128]] + list(lnb.ap)))

    # weights resident in SBUF (after x: LN1 must not wait behind these)
    wq_sb = const.tile([128, NCC, 512], BF16, tag="wq")
    wk_sb = const.tile([128, NCC, 512], BF16, tag="wk")
    wv_sb = const.tile([128, NCC, 512], BF16, tag="wv")
    wo_sb = const.tile([128, 4, C], BF16, tag="wo")
    nc.sync.dma_start(out=wq_sb, in_=wq)
    nc.sync.dma_start(out=wk_sb, in_=wk)
    nc.sync.dma_start(out=wv_sb, in_=wv)
    nc.sync.dma_start(out=wo_sb, in_=wo)

    hT = hTp.tile([128, NCC, T], BF16, tag="hT")

    # ---- Phase A: LN1 (own batch only) + transpose, 2 groups of 4 ----
    with tc.tile_pool(name="pst", bufs=2, space="PSUM") as PST:
        for grp in range(2):
            mvs = stat.tile([128, 4, 2], F32, tag="mvs", name=f"mvs_{grp}")
            rstd = stat.tile([128, 4], F32, tag="rstd", name=f"rstd_{grp}")
            lnv = stat.tile([128, 4], F32, tag="lnv", name=f"lnv_{grp}")
            for j in range(4):
                i = grp * 4 + j
                st = stat.tile([128, 2, 6], F32, tag="bn", name=f"bn_{i}")
                for k in range(2):
                    nc.vector.bn_stats(out=st[:, k, :],
                                       in_=xts[i][:, k * 512:(k + 1) * 512])
                nc.vector.bn_aggr(out=mvs[:, j, :], in_=st)
            nc.scalar.activation(out=lnv, in_=mvs[:, :, 1], func=AF.Ln,
                                 bias=eps_t)
            nc.scalar.activation(out=rstd, in_=lnv, func=AF.Exp, scale=-0.5)
            for j in range(4):
                i = grp * 4 + j
                ht = hp.tile([128, C], BF16, tag="h", name=f"h_{i}")
                nc.vector.tensor_scalar(
                    out=ht, in0=xts[i], scalar1=mvs[:, j, 0:1],
                    scalar2=rstd[:, j:j + 1], op0=ALU.subtract, op1=ALU.mult)
                if general_ln:
                    nc.vector.tensor_mul(out=ht, in0=ht, in1=lnw_bc)
                    nc.vector.tensor_add(out=ht, in0=ht, in1=lnb_bc)
                for g in range(2):
                    pt = PST.tile([128, 512], BF16, tag="tr",
                                  name=f"pt_{i}_{g}")
                    for c in range(4):
                        cc = g * 4 + c
                        nc.tensor.transpose(
                            pt[:, c * 128:(c + 1) * 128],
                            ht[:, cc * 128:(cc + 1) * 128], ident)
                    for c in range(4):
                        cc = g * 4 + c
                        nc.vector.tensor_copy(
                            out=hT[:, cc, i * 128:(i + 1) * 128],
                            in_=pt[:, c * 128:(c + 1) * 128])
    hp_cm.__exit__(None, None, None)
    xp_cm.__exit__(None, None, None)
    dnp = ctx.enter_context(tc.tile_pool(name="dnp", bufs=8))
    lnp = ctx.enter_context(tc.tile_pool(name="lnp", bufs=8))
    recp = ctx.enter_context(tc.tile_pool(name="recp", bufs=8))

    # ---- Phase B+C+D interleaved: qkv, scores+exp, av ----
    qT = [qTp.tile([128, T], BF16, tag="qT", name=f"qT_{m}")
          for m in range(4)]
    kT = [kTp.tile([128, T], BF16, tag="kT", name=f"kT_{m}")
          for m in range(4)]
    # v2[tok, sc, h, 0:64] = v; col 64 = ones (softmax denominator)
    v2 = v2p.tile([128, NTB, HPC, 66], BF16, tag="v2")
    nc.vector.memset(v2[:, :, :, 64:65], 1.0)
    nc.vector.memset(v2[:, :, :, 65:66], 0.0)
    cat = [ctp.tile([128, T], BF16, tag="cat", name=f"cat_{m}")
           for m in range(4)]
    # per-head softmax denominator rows and their reciprocals
    dens = [dnp.tile([1, T], BF16, tag="den", name=f"den_{h}")
            for h in range(HPC)]
    lnds = [lnp.tile([1, T], F32, tag="lnd", name=f"lnd_{h}")
            for h in range(HPC)]
    recs = [recp.tile([1, T], BF16, tag="rec", name=f"rec_{h}")
            for h in range(HPC)]
    pav_sbs = [avp.tile([128, T], F32R, tag="avsb", name=f"avsb_{m}")
               for m in range(4)]

    # PSUM budget: PSQ 1x[128,512]=1 bank, PSS 3x[128,<=512]=3,
    # PSA 2x[66,1024]=4  -> 8 banks total.
    with tc.tile_pool(name="psq", bufs=1, space="PSUM") as PSQ, \
         tc.tile_pool(name="pss", bufs=3, space="PSUM") as PSS, \
         tc.tile_pool(name="psa", bufs=2, space="PSUM") as PSA:

        def qkv_m(m):
            for th in range(2):
                tsl = slice(th * 512, (th + 1) * 512)
                pq = PSQ.tile([128, 512], F32, tag="mm", name=f"pq_{m}_{th}")
                for cc in range(NCC):
                    nc.tensor.matmul(pq, wq_sb[:, cc, m * 128:(m + 1) * 128],
                                     hT[:, cc, tsl],
                                     start=(cc == 0), stop=(cc == NCC - 1))
                nc.vector.tensor_copy(out=qT[m][:, tsl], in_=pq)
                pk = PSQ.tile([128, 512], F32, tag="mm", name=f"pk_{m}_{th}")
                for cc in range(NCC):
                    nc.tensor.matmul(pk, wk_sb[:, cc, m * 128:(m + 1) * 128],
                                     hT[:, cc, tsl],
                                     start=(cc == 0), stop=(cc == NCC - 1))
                nc.vector.tensor_copy(out=kT[m][:, tsl], in_=pk)

        def v_tb(tb):
            pv = PSQ.tile([128, 512], F32, tag="mm", name=f"pv_{tb}")
            for cc in range(NCC):
                nc.tensor.matmul(pv, hT[:, cc, tb * 128:(tb + 1) * 128],
                                 wv_sb[:, cc, :],
                                 start=(cc == 0), stop=(cc == NCC - 1))
            nc.vector.tensor_copy(
                out=v2[:, tb, :, 0:64],
                in_=bass.AP(tensor=pv.tensor, offset=pv.offset,
                            ap=list(pv.ap[:1]) + [[64, HPC], [1, 64]]))

        def scores_h(h):
            """scoresT + exp for head h; returns e tiles per sc."""
            m, ho = h // 2, (h % 2) * 64
            qh = qT[m][ho:ho + 64, :]
            kh = kT[m][ho:ho + 64, :]
            es = []
            for sc in range(NTB):
                W = T - sc * 128
                e = ep.tile([128, W], BF16, tag=f"e{sc}", name=f"e_{h}_{sc}")
                for (o0, o1) in _bank_chunks(0, W):
                    ps = PSS.tile([128, o1 - o0], F32, tag="sc",
                                  name=f"ps_{h}_{sc}_{o0}")
                    nc.tensor.matmul(
                        ps,
                        qh[:, sc * 128:(sc + 1) * 128],
                        kh[:, sc * 128 + o0:sc * 128 + o1],
                        start=True, stop=True)
                    if o0 == 0:
                        nc.vector.tensor_add(out=ps[:, 0:128],
                                             in0=ps[:, 0:128], in1=trilT)
                    nc.scalar.activation(out=e[:, o0:o1], in_=ps,
                                         func=AF.Exp, scale=SCALE)
                es.append(e)
            return es

        def av_h(h, es):
            """av for head h into pav_sb half + stash denominator row."""
            pav = PSA.tile([66, T], F32, tag="av", name=f"pav_{h}")
            for sc in range(NTB):
                lo = sc * 128
                for (o0, o1) in _bank_chunks(lo, T):
                    nc.tensor.matmul(
                        pav[:, o0:o1], v2[:, sc, h, :],
                        es[sc][:, o0 - lo:o1 - lo],
                        start=(sc == 0), stop=(sc == NTB - 1),
                        skip_group_check=True)
            ho = (h % 2) * 64
            nc.vector.tensor_copy(out=pav_sbs[h // 2][ho:ho + 64, :],
                                  in_=pav[0:64, :])
            nc.scalar.copy(out=dens[h], in_=pav[64:65, :])

        es0 = qkv_m(0) or scores_h(0)
        qkv_m(1)
        for tb in range(NTB):
            v_tb(tb)
        es1 = scores_h(1)
        qkv_m(2)
        es2 = scores_h(2)
        qkv_m(3)
        es3 = scores_h(3)

        es = {0: es0, 1: es1, 2: es2, 3: es3}
        for h in range(HPC):
            if h + 4 < HPC:
                es[h + 4] = scores_h(h + 4)
            av_h(h, es.pop(h))

    # ---- Phase E: 1/den via exp(-ln(den)), normalize, Wo projection ----
    with tc.tile_pool(name="psb", bufs=2, space="PSUM") as PSB, \
         tc.tile_pool(name="psp", bufs=2, space="PSUM") as PSP:
        with nc.allow_low_precision(reason="softmax denominator recip"):
            for grp in range(2):
                for i in range(4):
                    nc.scalar.activation(out=lnds[grp * 4 + i],
                                         in_=dens[grp * 4 + i], func=AF.Ln)
                for i in range(4):
                    nc.scalar.activation(out=recs[grp * 4 + i],
                                         in_=lnds[grp * 4 + i],
                                         func=AF.Exp, scale=-1.0)
        for m in range(4):
            # partition-broadcast each head's 1/den row via K=1 PE matmul
            prec = PSB.tile([128, T], F32, tag="prec", name=f"prec_{m}")
            for hh in range(2):
                for co in range(2):
                    nc.tensor.matmul(
                        prec[hh * 64:(hh + 1) * 64, co * 512:(co + 1) * 512],
                        ones64,
                        recs[2 * m + hh][:, co * 512:(co + 1) * 512],
                        start=True, stop=True)
            nc.vector.tensor_mul(out=cat[m], in0=pav_sbs[m], in1=prec)

        for tb in range(NTB):
            pp = PSP.tile([128, C], F32, tag="pp", name=f"pp_{tb}")
            for m in range(4):
                for co in range(2):
                    nc.tensor.matmul(
                        pp[:, co * 512:(co + 1) * 512],
                        cat[m][:, tb * 128:(tb + 1) * 128],
                        wo_sb[:, m, co * 512:(co + 1) * 512],
                        start=(m == 0), stop=(m == 3))
            o_sb = osp.tile([128, C], BF16, tag="o", name=f"o_{tb}")
            nc.vector.tensor_copy(out=o_sb, in_=pp)
            nc.sync.dma_start(out=pout[tb * 128:(tb + 1) * 128, :], in_=o_sb)


def _build_attn(general_ln: bool):
    nc = bacc.Bacc("TRN2", target_bir_lowering=False, debug=False)
    x = nc.dram_tensor("x", [T, C], BF16, kind="ExternalInput").ap()
    wq = nc.dram_tensor("wq", [128, NCC, 512], BF16, kind="ExternalInput").ap()
    wk = nc.dram_tensor("wk", [128, NCC, 512], BF16, kind="ExternalInput").ap()
    wv = nc.dram_tensor("wv", [128, NCC, 512], BF16, kind="ExternalInput").ap()
    wo = nc.dram_tensor("wo", [128, 4, C], BF16, kind="ExternalInput").ap()
    lnw = lnb = None
    if general_ln:
        lnw = nc.dram_tensor("lnw", [C], F32, kind="ExternalInput").ap()
        lnb = nc.dram_tensor("lnb", [C], F32, kind="ExternalInput").ap()
    ones_dram = nc.dram_tensor("ones", [1, 64], BF16,
                               kind="ExternalInput").ap()
    pout = nc.dram_tensor("pout", [T, C], BF16, kind="ExternalOutput").ap()
    with tile.TileContext(nc) as tc:
        with ExitStack() as ctx:
            _attn_body(ctx, tc, x, wq, wk, wv, wo, lnw, lnb, ones_dram, pout)
    nc.compile()
    return nc


# --------------------------------------------------------------------------
# kernel B: FFN, 512 rows per core
# --------------------------------------------------------------------------

def _ffn_body(ctx, tc, x2, w1, w2, b1, lnw, lnb, alpha, out):
    nc = tc.nc
    general_ln = lnw is not None

    const = ctx.enter_context(tc.tile_pool(name="const", bufs=1))
    xp = ctx.enter_context(tc.tile_pool(name="xp", bufs=NRB))
    # x2 tiles first: these DMAs gate the LN2 critical path
    x2ts = []
    for r in range(NRB):
        xt = xp.tile([128, C], F32, tag="x", name=f"x_{r}")
        nc.sync.dma_start(out=xt, in_=x2[r * 128:(r + 1) * 128, :])
        x2ts.append(xt)
    scratch = const.tile([128, 128], F32)
    make_identity(nc, scratch)
    ident = const.tile([128, 128], BF16)
    nc.vector.tensor_copy(out=ident, in_=scratch)
    eps_t = const.tile([128, 1], F32)
    nc.vector.memset(eps_t, EPS)
    if general_ln:
        lnw_bc = const.tile([128, C], F32, tag="lnw")
        lnb_bc = const.tile([128, C], F32, tag="lnb")
        nc.sync.dma_start(
            out=lnw_bc,
            in_=bass.AP(tensor=lnw.tensor, offset=lnw.offset,
                        ap=[[0, 128]] + list(lnw.ap)))
        nc.sync.dma_start(
            out=lnb_bc,
            in_=bass.AP(tensor=lnb.tensor, offset=lnb.offset,
                        ap=[[0, 128]] + list(lnb.ap)))
    b1_sb = None
    if b1 is not None:
        b1_sb = const.tile([128, NHID], F32, tag="b1")
        nc.sync.dma_start(out=b1_sb, in_=b1.rearrange("(h p) -> p h", p=128))

    # weights: big resident tiles, streamed in chunks of 8 hidden blocks
    w1_sb = const.tile([128, NHID, NCC, 128], BF16, tag="w1")
    w2_sb = const.tile([128, NHID, C], BF16, tag="w2")
    for hg in range(4):
        hsl = slice(hg * 8, (hg + 1) * 8)
        nc.sync.dma_start(out=w1_sb[:, hsl, :, :], in_=w1[:, hsl, :, :])
    for hg in range(4):
        hsl = slice(hg * 8, (hg + 1) * 8)
        nc.sync.dma_start(out=w2_sb[:, hsl, :], in_=w2[:, hsl, :])

    hp = ctx.enter_context(tc.tile_pool(name="hp", bufs=5))
    hTp = ctx.enter_context(tc.tile_pool(name="hTp", bufs=1))
    stat = ctx.enter_context(tc.tile_pool(name="stat", bufs=2))
    ftp = ctx.enter_context(tc.tile_pool(name="ftp", bufs=1))
    tmp = ctx.enter_context(tc.tile_pool(name="tmp", bufs=2))
    osp = ctx.enter_context(tc.tile_pool(name="osp", bufs=2))

    h2T = hTp.tile([128, NCC, RPC], BF16, tag="h2T")

    # ---- LN2 + transpose ----
    with tc.tile_pool(name="pst", bufs=2, space="PSUM") as PST:
        mvs = stat.tile([128, NRB, 2], F32, tag="mvs")
        rstd = stat.tile([128, NRB], F32, tag="rstd")
        lnv = stat.tile([128, NRB], F32, tag="lnv")
        for r in range(NRB):
            st = stat.tile([128, 2, 6], F32, tag="bn", name=f"bn_{r}")
            for k in range(2):
                nc.vector.bn_stats(out=st[:, k, :],
                                   in_=x2ts[r][:, k * 512:(k + 1) * 512])
            nc.vector.bn_aggr(out=mvs[:, r, :], in_=st)
        nc.scalar.activation(out=lnv, in_=mvs[:, :, 1], func=AF.Ln,
                             bias=eps_t)
        nc.scalar.activation(out=rstd, in_=lnv, func=AF.Exp, scale=-0.5)
        hts = []
        for r in range(NRB):
            ht = hp.tile([128, C], BF16, tag="h", name=f"h_{r}")
            nc.vector.tensor_scalar(
                out=ht, in0=x2ts[r], scalar1=mvs[:, r, 0:1],
                scalar2=rstd[:, r:r + 1], op0=ALU.subtract, op1=ALU.mult)
            if general_ln:
                nc.vector.tensor_mul(out=ht, in0=ht, in1=lnw_bc)
                nc.vector.tensor_add(out=ht, in0=ht, in1=lnb_bc)
            hts.append(ht)
        for cc in range(NCC):
            pt = PST.tile([128, RPC], BF16, tag="tr", name=f"pt_{cc}")
            for r in range(NRB):
                nc.tensor.transpose(
                    pt[:, r * 128:(r + 1) * 128],
                    hts[r][:, cc * 128:(cc + 1) * 128], ident)
            nc.vector.tensor_copy(out=h2T[:, cc, :], in_=pt)

    # ---- W1 + PReLU ----
    fbig = ftp.tile([128, NHID, RPC], BF16, tag="ft")
    with tc.tile_pool(name="psf", bufs=2, space="PSUM") as PSF:
        for h in range(NHID):
            pf = PSF.tile([128, RPC], F32, tag="f", name=f"pf_{h}")
            for cc in range(NCC):
                nc.tensor.matmul(pf, w1_sb[:, h, cc, :], h2T[:, cc, :],
                                 start=(cc == 0), stop=(cc == NCC - 1))
            if b1_sb is not None:
                nc.vector.tensor_scalar_add(out=pf, in0=pf,
                                            scalar1=b1_sb[:, h:h + 1])
            t1 = tmp.tile([128, RPC], F32, tag="t1", name=f"t1_{h}")
            nc.vector.tensor_scalar(
                out=t1, in0=pf, scalar1=0.0, scalar2=alpha - 1.0,
                op0=ALU.min, op1=ALU.mult)
            nc.vector.tensor_add(out=fbig[:, h, :], in0=pf, in1=t1)

    # ---- W2 + residual ----
    with tc.tile_pool(name="pso", bufs=2, space="PSUM") as PSO:
        for tb in range(NRB):
            po = PSO.tile([128, C], F32, tag="o", name=f"po_{tb}")
            for h in range(NHID):
                for co in range(2):
                    nc.tensor.matmul(
                        po[:, co * 512:(co + 1) * 512],
                        fbig[:, h, tb * 128:(tb + 1) * 128],
                        w2_sb[:, h, co * 512:(co + 1) * 512],
                        start=(h == 0), stop=(h == NHID - 1))
            o_sb = osp.tile([128, C], F32, tag="osb", name=f"osb_{tb}")
            nc.vector.tensor_add(out=o_sb, in0=po, in1=x2ts[tb])
            nc.sync.dma_start(out=out[tb * 128:(tb + 1) * 128, :], in_=o_sb)


def _build_ffn(general_ln: bool, has_b1: bool, alpha: float):
    nc = bacc.Bacc("TRN2", target_bir_lowering=False, debug=False)
    x2 = nc.dram_tensor("x2", [RPC, C], F32, kind="ExternalInput").ap()
    w1 = nc.dram_tensor("w1", [128, NHID, NCC, 128], BF16,
                        kind="ExternalInput").ap()
    w2 = nc.dram_tensor("w2", [128, NHID, C], BF16,
                        kind="ExternalInput").ap()
    b1 = lnw = lnb = None
    if has_b1:
        b1 = nc.dram_tensor("b1", [4 * C], F32, kind="ExternalInput").ap()
    if general_ln:
        lnw = nc.dram_tensor("lnw", [C], F32, kind="ExternalInput").ap()
        lnb = nc.dram_tensor("lnb", [C], F32, kind="ExternalInput").ap()
    out = nc.dram_tensor("out", [RPC, C], F32, kind="ExternalOutput").ap()
    with tile.TileContext(nc) as tc:
        with ExitStack() as ctx:
            _ffn_body(ctx, tc, x2, w1, w2, b1, lnw, lnb, alpha, out)
    nc.compile()
    return nc


# --------------------------------------------------------------------------
# host orchestration
# --------------------------------------------------------------------------

_NC_CACHE = {}

# bench-only instrumentation: when KBENCH_TRACE is set, launches run with
# trace=True and per-launch device exec_time_ns is appended here.
_TRACE = bool(os.environ.get("KBENCH_TRACE"))
EXEC_NS = []
TRACE_PATHS = []


def _run_spmd(nc, in_maps):
    res = run_bass_kernel_spmd(nc, in_maps, list(range(NCORES)),
                               trace=_TRACE,
                               trace_cores=list(range(NCORES)) if _TRACE
                               else None)
    if _TRACE:
        EXEC_NS.append(res.exec_time_ns)
        if res.instructions_and_trace is not None:
            TRACE_PATHS.append(res.instructions_and_trace[1])
    return res


def _bf16(a):
    import ml_dtypes
    return np.ascontiguousarray(np.asarray(a, np.float32)
                                .astype(ml_dtypes.bfloat16))


def _get_attn_nc(general_ln):
    key = ("attn", general_ln)
    if key not in _NC_CACHE:
        _NC_CACHE[key] = _build_attn(general_ln)
    return _NC_CACHE[key]


def _get_ffn_nc(general_ln, has_b1, alpha):
    key = ("ffn", general_ln, has_b1, float(alpha))
    if key not in _NC_CACHE:
        _NC_CACHE[key] = _build_ffn(general_ln, has_b1, float(alpha))
    return _NC_CACHE[key]


def _attn_weights(Wq, Wk, Wv, Wo):
    """Per-core weight arrays in the device layouts."""
    per_core = []
    for c in range(NCORES):
        hh = c % 2
        h0 = HPC * hh
        # [C, 512] -> [128, NCC, 512]
        def wlay(Wx):
            catw = np.concatenate([Wx[h] for h in range(h0, h0 + HPC)],
                                  axis=1)  # [C, 512]
            return _bf16(catw.reshape(NCC, 128, 512).transpose(1, 0, 2))
        wo = _bf16(Wo[hh * 512:(hh + 1) * 512].reshape(4, 128, C)
                   .transpose(1, 0, 2))
        per_core.append((wlay(Wq), wlay(Wk), wlay(Wv), wo))
    return per_core


def run_attn(x_flat, Wq, Wk, Wv, Wo, ln1_w, ln1_b):
    """Returns list of per-core partial projections [T, C] f32."""
    trivial = bool(np.all(ln1_w == 1.0) and np.all(ln1_b == 0.0))
    nc = _get_attn_nc(not trivial)
    wts = _attn_weights(Wq, Wk, Wv, Wo)
    in_maps = []
    for c in range(NCORES):
        b = c // 2
        wq, wk, wv, wo = wts[c]
        import ml_dtypes
        m = {"x": _bf16(x_flat[b * T:(b + 1) * T]),
             "wq": wq, "wk": wk, "wv": wv, "wo": wo,
             "ones": np.ones((1, 64), ml_dtypes.bfloat16)}
        if not trivial:
            m["lnw"] = np.asarray(ln1_w, np.float32)
            m["lnb"] = np.asarray(ln1_b, np.float32)
        in_maps.append(m)
    res = _run_spmd(nc, in_maps)
    return [res.results[c]["pout"] for c in range(NCORES)]


def run_ffn(x2_flat, W1, b1, W2, ln2_w, ln2_b, alpha):
    trivial = bool(np.all(ln2_w == 1.0) and np.all(ln2_b == 0.0))
    has_b1 = bool(np.any(b1 != 0.0))
    nc = _get_ffn_nc(not trivial, has_b1, alpha)
    w1l = _bf16(np.asarray(W1, np.float32)
                .reshape(NCC, 128, NHID, 128).transpose(1, 2, 0, 3))
    w2l = _bf16(np.asarray(W2, np.float32)
                .reshape(NHID, 128, C).transpose(1, 0, 2))
    in_maps = []
    for c in range(NCORES):
        m = {"x2": np.ascontiguousarray(x2_flat[RPC * c:RPC * (c + 1)]),
             "w1": w1l, "w2": w2l}
        if has_b1:
            m["b1"] = np.asarray(b1, np.float32)
        if not trivial:
            m["lnw"] = np.asarray(ln2_w, np.float32)
            m["lnb"] = np.asarray(ln2_b, np.float32)
        in_maps.append(m)
    res = _run_spmd(nc, in_maps)
    return np.concatenate(
        [res.results[c]["out"] for c in range(NCORES)], axis=0)


def kernel(x, ln1_w, ln1_b, Wk, Wq, Wv, Wo, bo, ln2_w, ln2_b, W1, b1,
           prelu_a, W2, b2):
    x = np.asarray(x, np.float32)
    x_flat = np.ascontiguousarray(x.reshape(B * T, C))
    alpha = float(np.asarray(prelu_a))

    parts = run_attn(x_flat, np.asarray(Wq, np.float32),
                     np.asarray(Wk, np.float32),
                     np.asarray(Wv, np.float32),
                     np.asarray(Wo, np.float32),
                     np.asarray(ln1_w, np.float32),
                     np.asarray(ln1_b, np.float32))
    # host reduction: x2 = x + partial_even + partial_odd (+ bo)
    x2 = np.empty_like(x_flat)
    for b in range(B):
        x2[b * T:(b + 1) * T] = (x_flat[b * T:(b + 1) * T]
                                 + parts[2 * b].astype(np.float32)
                                 + parts[2 * b + 1].astype(np.float32))
    bo = np.asarray(bo, np.float32)
    if np.any(bo != 0.0):
        x2 += bo
    out = run_ffn(x2, W1, np.asarray(b1, np.float32), W2,
                  np.asarray(ln2_w, np.float32),
                  np.asarray(ln2_b, np.float32), alpha)
    b2 = np.asarray(b2, np.float32)
    if np.any(b2 != 0.0):
        out = out + b2
    return out.reshape(B, T, C).astype(np.float32)


# revision 40
# speedup vs baseline: 1.0916x; 1.0181x over previous
"""Trainium2 Bass kernel for a dense pre-LN transformer block.

B=4, T=1024, C=1024, H=16 heads (head_size 64).

Distribution over 8 NeuronCores, two SPMD launches with a free host-side
reduction between them:

  Launch A (attention): core c works on batch b=c//2 and head-half
  hh=c%2 (8 heads). It computes LN1 for its batch only, projects
  q/k/v for its heads, runs causal softmax(k@q^T)-attention in the
  transposed-scores layout, and multiplies by its slice of Wo rows,
  producing a PARTIAL projection [T, C] (f32) for its batch.

  Host: x2[b] = x[b] + part[2b] + part[2b+1] (+bo).

  Launch B (FFN): core c runs LN2 + W1/PReLU/W2 + residual on rows
  [512c, 512(c+1)) of x2.

Matmul dtype strategy: the PE cost depends only on the MOVING operand
dtype and its free size (1 cycle/row for bf16 at any N, f32r at N>=256).
Activations that move (hT, wo, w2 stream, e) stay f32r/bf16 chosen for
SBUF fit; weights that sit stationary are bf16 (0.4% quantization).
Accumulation is always f32 in PSUM.
"""

import os
from contextlib import ExitStack

import numpy as np

import concourse.bass as bass
import concourse.tile as tile
from concourse import bacc, mybir
from concourse.bass_utils import run_bass_kernel_spmd
from concourse.masks import make_identity

F32 = mybir.dt.float32
F32R = mybir.dt.float32r
BF16 = mybir.dt.bfloat16
FP8 = mybir.dt.float8e4
AF = mybir.ActivationFunctionType
ALU = mybir.AluOpType

B, T, C, H, HS = 4, 1024, 1024, 16, 64
NCORES = 8
EPS = 1e-5
SCALE = float(C) ** -0.5  # folded into the softmax exp
NEG = -1e30

NTB = T // 128   # 8 token blocks per batch
NCC = C // 128   # 8 channel chunks
HPC = H // 2     # 8 heads per core
RPC = (B * T) // NCORES  # 512 rows per core in launch B
NRB = RPC // 128         # 4 row blocks
NHID = 4 * C // 128      # 32 hidden chunks


def _bank_chunks(lo, hi):
    """Split [lo, hi) at 512-column PSUM bank boundaries."""
    out = []
    o = lo
    while o < hi:
        n = min(512 - (o % 512), hi - o)
        out.append((o, o + n))
        o += n
    return out


# --------------------------------------------------------------------------
# kernel A: attention, one batch + 8 heads per core
# --------------------------------------------------------------------------

def _attn_body(ctx, tc, x, wq, wk, wv, wo, lnw, lnb, ones_dram, pout):
    nc = tc.nc
    general_ln = lnw is not None

    const = ctx.enter_context(tc.tile_pool(name="const", bufs=1))
    hTp = ctx.enter_context(tc.tile_pool(name="hTp", bufs=1))
    qTp = ctx.enter_context(tc.tile_pool(name="qTp", bufs=4))
    kTp = ctx.enter_context(tc.tile_pool(name="kTp", bufs=4))
    v2p = ctx.enter_context(tc.tile_pool(name="v2p", bufs=1))
    stat = ctx.enter_context(tc.tile_pool(name="stat", bufs=2))
    ep = ctx.enter_context(tc.tile_pool(name="ep", bufs=3))
    avp = ctx.enter_context(tc.tile_pool(name="avp", bufs=4))
    ctp = ctx.enter_context(tc.tile_pool(name="ctp", bufs=4))
    osp = ctx.enter_context(tc.tile_pool(name="osp", bufs=2))

    xp_cm = tc.tile_pool(name="xp", bufs=8)
    hp_cm = tc.tile_pool(name="hp", bufs=3)
    xp = xp_cm.__enter__()
    hp = hp_cm.__enter__()
    # x tiles first: these DMAs gate the LN1 critical path
    xts = []
    for i in range(NTB):
        xt = xp.tile([128, C], BF16, tag="x", name=f"x_{i}")
        nc.sync.dma_start(out=xt, in_=x[i * 128:(i + 1) * 128, :])
        xts.append(xt)

    scratch = const.tile([128, 128], F32)
    make_identity(nc, scratch)
    ident = const.tile([128, 128], BF16)
    nc.vector.tensor_copy(out=ident, in_=scratch)
    eps_t = const.tile([128, 1], F32)
    nc.vector.memset(eps_t, EPS)
    ones64 = const.tile([1, 64], BF16)
    nc.sync.dma_start(out=ones64, in_=ones_dram)
    if general_ln:
        lnw_bc = const.tile([128, C], F32, tag="lnw")
        lnb_bc = const.tile([128, C], F32, tag="lnb")
        nc.sync.dma_start(
            out=lnw_bc,
            in_=bass.AP(tensor=lnw.tensor, offset=lnw.offset,
                        ap=[[0, 128]] + list(lnw.ap)))
        nc.sync.dma_start(
            out=lnb_bc,
            in_=bass.AP(tensor=lnb.tensor, offset=lnb.offset,
                        ap=[[0, 128]] + list(lnb.ap)))

    # weights resident in SBUF (after x: LN1 must not wait behind these)
    wq_sb = const.tile([128, NCC, 512], BF16, tag="wq")
    wk_sb = const.tile([128, NCC, 512], BF16, tag="wk")
    wv_sb = const.tile([128, NCC, 512], BF16, tag="wv")
    wo_sb = const.tile([128, 4, C], BF16, tag="wo")
    nc.sync.dma_start(out=wq_sb, in_=wq)
    nc.sync.dma_start(out=wk_sb, in_=wk)
    nc.sync.dma_start(out=wv_sb, in_=wv)
    nc.sync.dma_start(out=wo_sb, in_=wo)

    hT = hTp.tile([128, NCC, T], BF16, tag="hT")

    # ---- Phase A: LN1 (own batch only) + transpose, 4 groups of 2 ----
    with tc.tile_pool(name="pst", bufs=2, space="PSUM") as PST:
        for grp in range(4):
            mvs = stat.tile([128, 2, 2], F32, tag="mvs", name=f"mvs_{grp}")
            rstd = stat.tile([128, 2], F32, tag="rstd", name=f"rstd_{grp}")
            lnv = stat.tile([128, 2], F32, tag="lnv", name=f"lnv_{grp}")
            for j in range(2):
                i = grp * 2 + j
                st = stat.tile([128, 2, 6], F32, tag="bn", name=f"bn_{i}")
                for k in range(2):
                    nc.vector.bn_stats(out=st[:, k, :],
                                       in_=xts[i][:, k * 512:(k + 1) * 512])
                nc.vector.bn_aggr(out=mvs[:, j, :], in_=st)
            nc.scalar.activation(out=lnv, in_=mvs[:, :, 1], func=AF.Ln,
                                 bias=eps_t)
            nc.scalar.activation(out=rstd, in_=lnv, func=AF.Exp, scale=-0.5)
            for j in range(2):
                i = grp * 2 + j
                ht = hp.tile([128, C], BF16, tag="h", name=f"h_{i}")
                nc.vector.tensor_scalar(
                    out=ht, in0=xts[i], scalar1=mvs[:, j, 0:1],
                    scalar2=rstd[:, j:j + 1], op0=ALU.subtract, op1=ALU.mult)
                if general_ln:
                    nc.vector.tensor_mul(out=ht, in0=ht, in1=lnw_bc)
                    nc.vector.tensor_add(out=ht, in0=ht, in1=lnb_bc)
                for g in range(2):
                    pt = PST.tile([128, 512], BF16, tag="tr",
                                  name=f"pt_{i}_{g}")
                    for c in range(4):
                        cc = g * 4 + c
                        nc.tensor.transpose(
                            pt[:, c * 128:(c + 1) * 128],
                            ht[:, cc * 128:(cc + 1) * 128], ident)
                    for c in range(4):
                        cc = g * 4 + c
                        nc.vector.tensor_copy(
                            out=hT[:, cc, i * 128:(i + 1) * 128],
                            in_=pt[:, c * 128:(c + 1) * 128])
    hp_cm.__exit__(None, None, None)
    xp_cm.__exit__(None, None, None)
    dnp = ctx.enter_context(tc.tile_pool(name="dnp", bufs=8))
    lnp = ctx.enter_context(tc.tile_pool(name="lnp", bufs=8))
    recp = ctx.enter_context(tc.tile_pool(name="recp", bufs=8))

    # ---- Phase B+C+D interleaved: qkv, scores+exp, av ----
    qT = [qTp.tile([128, T], BF16, tag="qT", name=f"qT_{m}")
          for m in range(4)]
    kT = [kTp.tile([128, T], BF16, tag="kT", name=f"kT_{m}")
          for m in range(4)]
    # v2[tok, sc, h, 0:64] = v; col 64 = ones (softmax denominator)
    v2 = v2p.tile([128, NTB, HPC, 66], BF16, tag="v2")
    nc.vector.memset(v2[:, :, :, 64:65], 1.0)
    nc.vector.memset(v2[:, :, :, 65:66], 0.0)
    cat = [ctp.tile([128, T], BF16, tag="cat", name=f"cat_{m}")
           for m in range(4)]
    # per-head softmax denominator rows and their reciprocals
    dens = [dnp.tile([1, T], BF16, tag="den", name=f"den_{h}")
            for h in range(HPC)]
    lnds = [lnp.tile([1, T], F32, tag="lnd", name=f"lnd_{h}")
            for h in range(HPC)]
    recs = [recp.tile([1, T], BF16, tag="rec", name=f"rec_{h}")
            for h in range(HPC)]
    pav_sbs = [avp.tile([128, T], F32R, tag="avsb", name=f"avsb_{m}")
               for m in range(4)]

    # PSUM budget: PSQ 1x[128,512]=1 bank, PSS 3x[128,<=512]=3,
    # PSA 2x[66,1024]=4  -> 8 banks total.
    with tc.tile_pool(name="psq", bufs=1, space="PSUM") as PSQ, \
         tc.tile_pool(name="pss", bufs=3, space="PSUM") as PSS, \
         tc.tile_pool(name="psa", bufs=2, space="PSUM") as PSA:

        def qkv_m(m):
            for th in range(2):
                tsl = slice(th * 512, (th + 1) * 512)
                pq = PSQ.tile([128, 512], F32, tag="mm", name=f"pq_{m}_{th}")
                for cc in range(NCC):
                    nc.tensor.matmul(pq, wq_sb[:, cc, m * 128:(m + 1) * 128],
                                     hT[:, cc, tsl],
                                     start=(cc == 0), stop=(cc == NCC - 1))
                nc.vector.tensor_copy(out=qT[m][:, tsl], in_=pq)
                pk = PSQ.tile([128, 512], F32, tag="mm", name=f"pk_{m}_{th}")
                for cc in range(NCC):
                    nc.tensor.matmul(pk, wk_sb[:, cc, m * 128:(m + 1) * 128],
                                     hT[:, cc, tsl],
                                     start=(cc == 0), stop=(cc == NCC - 1))
                nc.vector.tensor_copy(out=kT[m][:, tsl], in_=pk)

        def v_tb(tb):
            pv = PSQ.tile([128, 512], F32, tag="mm", name=f"pv_{tb}")
            for cc in range(NCC):
                nc.tensor.matmul(pv, hT[:, cc, tb * 128:(tb + 1) * 128],
                                 wv_sb[:, cc, :],
                                 start=(cc == 0), stop=(cc == NCC - 1))
            nc.vector.tensor_copy(
                out=v2[:, tb, :, 0:64],
                in_=bass.AP(tensor=pv.tensor, offset=pv.offset,
                            ap=list(pv.ap[:1]) + [[64, HPC], [1, 64]]))

        def scores_h(h):
            """scoresT + exp for head h; returns e tiles per sc."""
            m, ho = h // 2, (h % 2) * 64
            qh = qT[m][ho:ho + 64, :]
            kh = kT[m][ho:ho + 64, :]
            es = []
            for sc in range(NTB):
                W = T - sc * 128
                e = ep.tile([128, W], BF16, tag=f"e{sc}", name=f"e_{h}_{sc}")
                for (o0, o1) in _bank_chunks(0, W):
                    ps = PSS.tile([128, o1 - o0], F32, tag="sc",
                                  name=f"ps_{h}_{sc}_{o0}")
                    nc.tensor.matmul(
                        ps,
                        qh[:, sc * 128:(sc + 1) * 128],
                        kh[:, sc * 128 + o0:sc * 128 + o1],
                        start=True, stop=True)
                    nc.scalar.activation(out=e[:, o0:o1], in_=ps,
                                         func=AF.Exp, scale=SCALE)
                # zero the upper triangle of the diagonal block (gpsimd,
                # off the PE->ACT critical chain)
                nc.gpsimd.affine_select(
                    out=e[:, 0:128], in_=e[:, 0:128], compare_op=ALU.is_ge,
                    fill=0.0, base=0, pattern=[[1, 128]],
                    channel_multiplier=-1)
                es.append(e)
            return es

        def av_h(h, es):
            """av for head h into pav_sb half + stash denominator row."""
            pav = PSA.tile([66, T], F32, tag="av", name=f"pav_{h}")
            for sc in range(NTB):
                lo = sc * 128
                for (o0, o1) in _bank_chunks(lo, T):
                    nc.tensor.matmul(
                        pav[:, o0:o1], v2[:, sc, h, :],
                        es[sc][:, o0 - lo:o1 - lo],
                        start=(sc == 0), stop=(sc == NTB - 1),
                        skip_group_check=True)
            ho = (h % 2) * 64
            nc.vector.tensor_copy(out=pav_sbs[h // 2][ho:ho + 64, :],
                                  in_=pav[0:64, :])
            nc.scalar.copy(out=dens[h], in_=pav[64:65, :])

        es0 = qkv_m(0) or scores_h(0)
        qkv_m(1)
        for tb in range(NTB):
            v_tb(tb)
        es1 = scores_h(1)
        qkv_m(2)
        es2 = scores_h(2)
        qkv_m(3)
        es3 = scores_h(3)

        es = {0: es0, 1: es1, 2: es2, 3: es3}
        for h in range(HPC):
            if h + 4 < HPC:
                es[h + 4] = scores_h(h + 4)
            av_h(h, es.pop(h))

    # ---- Phase E: 1/den via exp(-ln(den)), normalize, Wo projection ----
    with tc.tile_pool(name="psb", bufs=2, space="PSUM") as PSB, \
         tc.tile_pool(name="psp", bufs=2, space="PSUM") as PSP:
        with nc.allow_low_precision(reason="softmax denominator recip"):
            for grp in range(2):
                for i in range(4):
                    nc.scalar.activation(out=lnds[grp * 4 + i],
                                         in_=dens[grp * 4 + i], func=AF.Ln)
                for i in range(4):
                    nc.scalar.activation(out=recs[grp * 4 + i],
                                         in_=lnds[grp * 4 + i],
                                         func=AF.Exp, scale=-1.0)
        for m in range(4):
            # partition-broadcast each head's 1/den row via K=1 PE matmul
            prec = PSB.tile([128, T], F32, tag="prec", name=f"prec_{m}")
            for hh in range(2):
                for co in range(2):
                    nc.tensor.matmul(
                        prec[hh * 64:(hh + 1) * 64, co * 512:(co + 1) * 512],
                        ones64,
                        recs[2 * m + hh][:, co * 512:(co + 1) * 512],
                        start=True, stop=True)
            nc.vector.tensor_mul(out=cat[m], in0=pav_sbs[m], in1=prec)

        for tb in range(NTB):
            pp = PSP.tile([128, C], F32, tag="pp", name=f"pp_{tb}")
            for m in range(4):
                for co in range(2):
                    nc.tensor.matmul(
                        pp[:, co * 512:(co + 1) * 512],
                        cat[m][:, tb * 128:(tb + 1) * 128],
                        wo_sb[:, m, co * 512:(co + 1) * 512],
                        start=(m == 0), stop=(m == 3))
            o_sb = osp.tile([128, C], BF16, tag="o", name=f"o_{tb}")
            nc.vector.tensor_copy(out=o_sb, in_=pp)
            nc.sync.dma_start(out=pout[tb * 128:(tb + 1) * 128, :], in_=o_sb)


def _build_attn(general_ln: bool):
    nc = bacc.Bacc("TRN2", target_bir_lowering=False, debug=False)
    x = nc.dram_tensor("x", [T, C], BF16, kind="ExternalInput").ap()
    wq = nc.dram_tensor("wq", [128, NCC, 512], BF16, kind="ExternalInput").ap()
    wk = nc.dram_tensor("wk", [128, NCC, 512], BF16, kind="ExternalInput").ap()
    wv = nc.dram_tensor("wv", [128, NCC, 512], BF16, kind="ExternalInput").ap()
    wo = nc.dram_tensor("wo", [128, 4, C], BF16, kind="ExternalInput").ap()
    lnw = lnb = None
    if general_ln:
        lnw = nc.dram_tensor("lnw", [C], F32, kind="ExternalInput").ap()
        lnb = nc.dram_tensor("lnb", [C], F32, kind="ExternalInput").ap()
    ones_dram = nc.dram_tensor("ones", [1, 64], BF16,
                               kind="ExternalInput").ap()
    pout = nc.dram_tensor("pout", [T, C], BF16, kind="ExternalOutput").ap()
    with tile.TileContext(nc) as tc:
        with ExitStack() as ctx:
            _attn_body(ctx, tc, x, wq, wk, wv, wo, lnw, lnb, ones_dram, pout)
    nc.compile()
    return nc


# --------------------------------------------------------------------------
# kernel B: FFN, 512 rows per core
# --------------------------------------------------------------------------

def _ffn_body(ctx, tc, x2, w1, w2, b1, lnw, lnb, alpha, out):
    nc = tc.nc
    general_ln = lnw is not None

    const = ctx.enter_context(tc.tile_pool(name="const", bufs=1))
    xp = ctx.enter_context(tc.tile_pool(name="xp", bufs=NRB))
    # x2 tiles first: these DMAs gate the LN2 critical path
    x2ts = []
    for r in range(NRB):
        xt = xp.tile([128, C], F32, tag="x", name=f"x_{r}")
        nc.sync.dma_start(out=xt, in_=x2[r * 128:(r + 1) * 128, :])
        x2ts.append(xt)
    scratch = const.tile([128, 128], F32)
    make_identity(nc, scratch)
    ident = const.tile([128, 128], BF16)
    nc.vector.tensor_copy(out=ident, in_=scratch)
    eps_t = const.tile([128, 1], F32)
    nc.vector.memset(eps_t, EPS)
    if general_ln:
        lnw_bc = const.tile([128, C], F32, tag="lnw")
        lnb_bc = const.tile([128, C], F32, tag="lnb")
        nc.sync.dma_start(
            out=lnw_bc,
            in_=bass.AP(tensor=lnw.tensor, offset=lnw.offset,
                        ap=[[0, 128]] + list(lnw.ap)))
        nc.sync.dma_start(
            out=lnb_bc,
            in_=bass.AP(tensor=lnb.tensor, offset=lnb.offset,
                        ap=[[0, 128]] + list(lnb.ap)))
    b1_sb = None
    if b1 is not None:
        b1_sb = const.tile([128, NHID], F32, tag="b1")
        nc.sync.dma_start(out=b1_sb, in_=b1.rearrange("(h p) -> p h", p=128))

    # weights: big resident tiles, streamed in chunks of 8 hidden blocks
    w1_sb = const.tile([128, NHID, NCC, 128], BF16, tag="w1")
    w2_sb = const.tile([128, NHID, C], BF16, tag="w2")
    for hg in range(4):
        hsl = slice(hg * 8, (hg + 1) * 8)
        nc.sync.dma_start(out=w1_sb[:, hsl, :, :], in_=w1[:, hsl, :, :])
    for hg in range(4):
        hsl = slice(hg * 8, (hg + 1) * 8)
        nc.sync.dma_start(out=w2_sb[:, hsl, :], in_=w2[:, hsl, :])

    hp = ctx.enter_context(tc.tile_pool(name="hp", bufs=5))
    hTp = ctx.enter_context(tc.tile_pool(name="hTp", bufs=1))
    stat = ctx.enter_context(tc.tile_pool(name="stat", bufs=2))
    ftp = ctx.enter_context(tc.tile_pool(name="ftp", bufs=1))
    tmp = ctx.enter_context(tc.tile_pool(name="tmp", bufs=2))
    osp = ctx.enter_context(tc.tile_pool(name="osp", bufs=2))

    h2T = hTp.tile([128, NCC, RPC], BF16, tag="h2T")

    # ---- LN2 + transpose ----
    with tc.tile_pool(name="pst", bufs=2, space="PSUM") as PST:
        hts = []
        for grp in range(2):
            mvs = stat.tile([128, 2, 2], F32, tag="mvs", name=f"mvs_{grp}")
            rstd = stat.tile([128, 2], F32, tag="rstd", name=f"rs_{grp}")
            lnv = stat.tile([128, 2], F32, tag="lnv", name=f"lv_{grp}")
            for j in range(2):
                r = grp * 2 + j
                st = stat.tile([128, 2, 6], F32, tag="bn", name=f"bn_{r}")
                for k in range(2):
                    nc.vector.bn_stats(
                        out=st[:, k, :],
                        in_=x2ts[r][:, k * 512:(k + 1) * 512])
                nc.vector.bn_aggr(out=mvs[:, j, :], in_=st)
            nc.scalar.activation(out=lnv, in_=mvs[:, :, 1], func=AF.Ln,
                                 bias=eps_t)
            nc.scalar.activation(out=rstd, in_=lnv, func=AF.Exp, scale=-0.5)
            for j in range(2):
                r = grp * 2 + j
                ht = hp.tile([128, C], BF16, tag="h", name=f"h_{r}")
                nc.vector.tensor_scalar(
                    out=ht, in0=x2ts[r], scalar1=mvs[:, j, 0:1],
                    scalar2=rstd[:, j:j + 1], op0=ALU.subtract, op1=ALU.mult)
                if general_ln:
                    nc.vector.tensor_mul(out=ht, in0=ht, in1=lnw_bc)
                    nc.vector.tensor_add(out=ht, in0=ht, in1=lnb_bc)
                hts.append(ht)
        for cc in range(NCC):
            pt = PST.tile([128, RPC], BF16, tag="tr", name=f"pt_{cc}")
            for r in range(NRB):
                nc.tensor.transpose(
                    pt[:, r * 128:(r + 1) * 128],
                    hts[r][:, cc * 128:(cc + 1) * 128], ident)
            nc.vector.tensor_copy(out=h2T[:, cc, :], in_=pt)

    # ---- W1 + PReLU ----
    fbig = ftp.tile([128, NHID, RPC], BF16, tag="ft")
    with tc.tile_pool(name="psf", bufs=2, space="PSUM") as PSF:
        for h in range(NHID):
            pf = PSF.tile([128, RPC], F32, tag="f", name=f"pf_{h}")
            for cc in range(NCC):
                nc.tensor.matmul(pf, w1_sb[:, h, cc, :], h2T[:, cc, :],
                                 start=(cc == 0), stop=(cc == NCC - 1))
            if b1_sb is not None:
                nc.vector.tensor_scalar_add(out=pf, in0=pf,
                                            scalar1=b1_sb[:, h:h + 1])
            t1 = tmp.tile([128, RPC], F32, tag="t1", name=f"t1_{h}")
            nc.vector.tensor_scalar(
                out=t1, in0=pf, scalar1=0.0, scalar2=alpha - 1.0,
                op0=ALU.min, op1=ALU.mult)
            nc.vector.tensor_add(out=fbig[:, h, :], in0=pf, in1=t1)

    # ---- W2 + residual ----
    with tc.tile_pool(name="pso", bufs=2, space="PSUM") as PSO:
        for tb in range(NRB):
            po = PSO.tile([128, C], F32, tag="o", name=f"po_{tb}")
            for h in range(NHID):
                for co in range(2):
                    nc.tensor.matmul(
                        po[:, co * 512:(co + 1) * 512],
                        fbig[:, h, tb * 128:(tb + 1) * 128],
                        w2_sb[:, h, co * 512:(co + 1) * 512],
                        start=(h == 0), stop=(h == NHID - 1))
            o_sb = osp.tile([128, C], F32, tag="osb", name=f"osb_{tb}")
            nc.vector.tensor_add(out=o_sb, in0=po, in1=x2ts[tb])
            nc.sync.dma_start(out=out[tb * 128:(tb + 1) * 128, :], in_=o_sb)


def _build_ffn(general_ln: bool, has_b1: bool, alpha: float):
    nc = bacc.Bacc("TRN2", target_bir_lowering=False, debug=False)
    x2 = nc.dram_tensor("x2", [RPC, C], F32, kind="ExternalInput").ap()
    w1 = nc.dram_tensor("w1", [128, NHID, NCC, 128], BF16,
                        kind="ExternalInput").ap()
    w2 = nc.dram_tensor("w2", [128, NHID, C], BF16,
                        kind="ExternalInput").ap()
    b1 = lnw = lnb = None
    if has_b1:
        b1 = nc.dram_tensor("b1", [4 * C], F32, kind="ExternalInput").ap()
    if general_ln:
        lnw = nc.dram_tensor("lnw", [C], F32, kind="ExternalInput").ap()
        lnb = nc.dram_tensor("lnb", [C], F32, kind="ExternalInput").ap()
    out = nc.dram_tensor("out", [RPC, C], F32, kind="ExternalOutput").ap()
    with tile.TileContext(nc) as tc:
        with ExitStack() as ctx:
            _ffn_body(ctx, tc, x2, w1, w2, b1, lnw, lnb, alpha, out)
    nc.compile()
    return nc


# --------------------------------------------------------------------------
# host orchestration
# --------------------------------------------------------------------------

_NC_CACHE = {}

# bench-only instrumentation: when KBENCH_TRACE is set, launches run with
# trace=True and per-launch device exec_time_ns is appended here.
_TRACE = bool(os.environ.get("KBENCH_TRACE"))
EXEC_NS = []
TRACE_PATHS = []


def _run_spmd(nc, in_maps):
    res = run_bass_kernel_spmd(nc, in_maps, list(range(NCORES)),
                               trace=_TRACE,
                               trace_cores=list(range(NCORES)) if _TRACE
                               else None)
    if _TRACE:
        EXEC_NS.append(res.exec_time_ns)
        if res.instructions_and_trace is not None:
            TRACE_PATHS.append(res.instructions_and_trace[1])
    return res


def _bf16(a):
    import ml_dtypes
    return np.ascontiguousarray(np.asarray(a, np.float32)
                                .astype(ml_dtypes.bfloat16))


def _get_attn_nc(general_ln):
    key = ("attn", general_ln)
    if key not in _NC_CACHE:
        _NC_CACHE[key] = _build_attn(general_ln)
    return _NC_CACHE[key]


def _get_ffn_nc(general_ln, has_b1, alpha):
    key = ("ffn", general_ln, has_b1, float(alpha))
    if key not in _NC_CACHE:
        _NC_CACHE[key] = _build_ffn(general_ln, has_b1, float(alpha))
    return _NC_CACHE[key]


def _attn_weights(Wq, Wk, Wv, Wo):
    """Per-core weight arrays in the device layouts."""
    per_core = []
    for c in range(NCORES):
        hh = c % 2
        h0 = HPC * hh
        # [C, 512] -> [128, NCC, 512]
        def wlay(Wx):
            catw = np.concatenate([Wx[h] for h in range(h0, h0 + HPC)],
                                  axis=1)  # [C, 512]
            return _bf16(catw.reshape(NCC, 128, 512).transpose(1, 0, 2))
        wo = _bf16(Wo[hh * 512:(hh + 1) * 512].reshape(4, 128, C)
                   .transpose(1, 0, 2))
        per_core.append((wlay(Wq), wlay(Wk), wlay(Wv), wo))
    return per_core


def run_attn(x_flat, Wq, Wk, Wv, Wo, ln1_w, ln1_b):
    """Returns list of per-core partial projections [T, C] f32."""
    trivial = bool(np.all(ln1_w == 1.0) and np.all(ln1_b == 0.0))
    nc = _get_attn_nc(not trivial)
    wts = _attn_weights(Wq, Wk, Wv, Wo)
    in_maps = []
    for c in range(NCORES):
        b = c // 2
        wq, wk, wv, wo = wts[c]
        import ml_dtypes
        m = {"x": _bf16(x_flat[b * T:(b + 1) * T]),
             "wq": wq, "wk": wk, "wv": wv, "wo": wo,
             "ones": np.ones((1, 64), ml_dtypes.bfloat16)}
        if not trivial:
            m["lnw"] = np.asarray(ln1_w, np.float32)
            m["lnb"] = np.asarray(ln1_b, np.float32)
        in_maps.append(m)
    res = _run_spmd(nc, in_maps)
    return [res.results[c]["pout"] for c in range(NCORES)]


def run_ffn(x2_flat, W1, b1, W2, ln2_w, ln2_b, alpha):
    trivial = bool(np.all(ln2_w == 1.0) and np.all(ln2_b == 0.0))
    has_b1 = bool(np.any(b1 != 0.0))
    nc = _get_ffn_nc(not trivial, has_b1, alpha)
    w1l = _bf16(np.asarray(W1, np.float32)
                .reshape(NCC, 128, NHID, 128).transpose(1, 2, 0, 3))
    w2l = _bf16(np.asarray(W2, np.float32)
                .reshape(NHID, 128, C).transpose(1, 0, 2))
    in_maps = []
    for c in range(NCORES):
        m = {"x2": np.ascontiguousarray(x2_flat[RPC * c:RPC * (c + 1)]),
             "w1": w1l, "w2": w2l}
        if has_b1:
            m["b1"] = np.asarray(b1, np.float32)
        if not trivial:
            m["lnw"] = np.asarray(ln2_w, np.float32)
            m["lnb"] = np.asarray(ln2_b, np.float32)
        in_maps.append(m)
    res = _run_spmd(nc, in_maps)
    return np.concatenate(
        [res.results[c]["out"] for c in range(NCORES)], axis=0)


def kernel(x, ln1_w, ln1_b, Wk, Wq, Wv, Wo, bo, ln2_w, ln2_b, W1, b1,
           prelu_a, W2, b2):
    x = np.asarray(x, np.float32)
    x_flat = np.ascontiguousarray(x.reshape(B * T, C))
    alpha = float(np.asarray(prelu_a))

    parts = run_attn(x_flat, np.asarray(Wq, np.float32),
                     np.asarray(Wk, np.float32),
                     np.asarray(Wv, np.float32),
                     np.asarray(Wo, np.float32),
                     np.asarray(ln1_w, np.float32),
                     np.asarray(ln1_b, np.float32))
    # host reduction: x2 = x + partial_even + partial_odd (+ bo)
    x2 = np.empty_like(x_flat)
    for b in range(B):
        x2[b * T:(b + 1) * T] = (x_flat[b * T:(b + 1) * T]
                                 + parts[2 * b].astype(np.float32)
                                 + parts[2 * b + 1].astype(np.float32))
    bo = np.asarray(bo, np.float32)
    if np.any(bo != 0.0):
        x2 += bo
    out = run_ffn(x2, W1, np.asarray(b1, np.float32), W2,
                  np.asarray(ln2_w, np.float32),
                  np.asarray(ln2_b, np.float32), alpha)
    b2 = np.asarray(b2, np.float32)
    if np.any(b2 != 0.0):
        out = out + b2
    return out.reshape(B, T, C).astype(np.float32)


# revision 43
# speedup vs baseline: 1.1013x; 1.0089x over previous
"""Trainium2 Bass kernel for a dense pre-LN transformer block.

B=4, T=1024, C=1024, H=16 heads (head_size 64).

Distribution over 8 NeuronCores, two SPMD launches with a free host-side
reduction between them:

  Launch A (attention): core c works on batch b=c//2 and head-half
  hh=c%2 (8 heads). It computes LN1 for its batch only, projects
  q/k/v for its heads, runs causal softmax(k@q^T)-attention in the
  transposed-scores layout, and multiplies by its slice of Wo rows,
  producing a PARTIAL projection [T, C] (f32) for its batch.

  Host: x2[b] = x[b] + part[2b] + part[2b+1] (+bo).

  Launch B (FFN): core c runs LN2 + W1/PReLU/W2 + residual on rows
  [512c, 512(c+1)) of x2.

Matmul dtype strategy: the PE cost depends only on the MOVING operand
dtype and its free size (1 cycle/row for bf16 at any N, f32r at N>=256).
Activations that move (hT, wo, w2 stream, e) stay f32r/bf16 chosen for
SBUF fit; weights that sit stationary are bf16 (0.4% quantization).
Accumulation is always f32 in PSUM.
"""

import os
from contextlib import ExitStack

import numpy as np

import concourse.bass as bass
import concourse.tile as tile
from concourse import bacc, mybir
from concourse.bass_utils import run_bass_kernel_spmd
from concourse.masks import make_identity

F32 = mybir.dt.float32
F32R = mybir.dt.float32r
BF16 = mybir.dt.bfloat16
FP8 = mybir.dt.float8e4
AF = mybir.ActivationFunctionType
ALU = mybir.AluOpType

B, T, C, H, HS = 4, 1024, 1024, 16, 64
NCORES = 8
EPS = 1e-5
SCALE = float(C) ** -0.5  # folded into the softmax exp
NEG = -1e30

NTB = T // 128   # 8 token blocks per batch
NCC = C // 128   # 8 channel chunks
HPC = H // 2     # 8 heads per core
RPC = (B * T) // NCORES  # 512 rows per core in launch B
NRB = RPC // 128         # 4 row blocks
NHID = 4 * C // 128      # 32 hidden chunks


def _bank_chunks(lo, hi):
    """Split [lo, hi) at 512-column PSUM bank boundaries."""
    out = []
    o = lo
    while o < hi:
        n = min(512 - (o % 512), hi - o)
        out.append((o, o + n))
        o += n
    return out


# --------------------------------------------------------------------------
# kernel A: attention, one batch + 8 heads per core
# --------------------------------------------------------------------------

def _attn_body(ctx, tc, x, wq, wk, wv, wo, lnw, lnb, ones_dram, pout):
    nc = tc.nc
    general_ln = lnw is not None

    const = ctx.enter_context(tc.tile_pool(name="const", bufs=1))
    hTp = ctx.enter_context(tc.tile_pool(name="hTp", bufs=1))
    qTp = ctx.enter_context(tc.tile_pool(name="qTp", bufs=4))
    kTp = ctx.enter_context(tc.tile_pool(name="kTp", bufs=4))
    v2p = ctx.enter_context(tc.tile_pool(name="v2p", bufs=1))
    stat = ctx.enter_context(tc.tile_pool(name="stat", bufs=2))
    ep = ctx.enter_context(tc.tile_pool(name="ep", bufs=3))
    avp = ctx.enter_context(tc.tile_pool(name="avp", bufs=4))
    ctp = ctx.enter_context(tc.tile_pool(name="ctp", bufs=4))
    osp = ctx.enter_context(tc.tile_pool(name="osp", bufs=2))

    xp_cm = tc.tile_pool(name="xp", bufs=8)
    hp_cm = tc.tile_pool(name="hp", bufs=3)
    xp = xp_cm.__enter__()
    hp = hp_cm.__enter__()
    # x tiles first: these DMAs gate the LN1 critical path
    xts = []
    for i in range(NTB):
        xt = xp.tile([128, C], BF16, tag="x", name=f"x_{i}")
        nc.sync.dma_start(out=xt, in_=x[i * 128:(i + 1) * 128, :])
        xts.append(xt)

    scratch = const.tile([128, 128], F32)
    make_identity(nc, scratch)
    ident = const.tile([128, 128], BF16)
    nc.vector.tensor_copy(out=ident, in_=scratch)
    eps_t = const.tile([128, 1], F32)
    nc.vector.memset(eps_t, EPS)
    ones64 = const.tile([1, 64], BF16)
    nc.sync.dma_start(out=ones64, in_=ones_dram)
    if general_ln:
        lnw_bc = const.tile([128, C], F32, tag="lnw")
        lnb_bc = const.tile([128, C], F32, tag="lnb")
        nc.sync.dma_start(
            out=lnw_bc,
            in_=bass.AP(tensor=lnw.tensor, offset=lnw.offset,
                        ap=[[0, 128]] + list(lnw.ap)))
        nc.sync.dma_start(
            out=lnb_bc,
            in_=bass.AP(tensor=lnb.tensor, offset=lnb.offset,
                        ap=[[0, 128]] + list(lnb.ap)))

    # weights resident in SBUF (after x: LN1 must not wait behind these)
    wq_sb = const.tile([128, NCC, 512], BF16, tag="wq")
    wk_sb = const.tile([128, NCC, 512], BF16, tag="wk")
    wv_sb = const.tile([128, NCC, 512], BF16, tag="wv")
    wo_sb = const.tile([128, 4, C], BF16, tag="wo")
    nc.sync.dma_start(out=wq_sb, in_=wq)
    nc.sync.dma_start(out=wk_sb, in_=wk)
    nc.sync.dma_start(out=wv_sb, in_=wv)
    nc.sync.dma_start(out=wo_sb, in_=wo)

    hT = hTp.tile([128, NCC, T], BF16, tag="hT")

    # ---- Phase A: LN1 (own batch only) + transpose, 4 groups of 2 ----
    with tc.tile_pool(name="pst", bufs=2, space="PSUM") as PST:
        for grp in range(4):
            mvs = stat.tile([128, 2, 2], F32, tag="mvs", name=f"mvs_{grp}")
            rstd = stat.tile([128, 2], F32, tag="rstd", name=f"rstd_{grp}")
            lnv = stat.tile([128, 2], F32, tag="lnv", name=f"lnv_{grp}")
            for j in range(2):
                i = grp * 2 + j
                st = stat.tile([128, 2, 6], F32, tag="bn", name=f"bn_{i}")
                for k in range(2):
                    nc.vector.bn_stats(out=st[:, k, :],
                                       in_=xts[i][:, k * 512:(k + 1) * 512])
                nc.vector.bn_aggr(out=mvs[:, j, :], in_=st)
            nc.scalar.activation(out=lnv, in_=mvs[:, :, 1], func=AF.Ln,
                                 bias=eps_t)
            nc.scalar.activation(out=rstd, in_=lnv, func=AF.Exp, scale=-0.5)
            for j in range(2):
                i = grp * 2 + j
                ht = hp.tile([128, C], BF16, tag="h", name=f"h_{i}")
                nc.vector.tensor_scalar(
                    out=ht, in0=xts[i], scalar1=mvs[:, j, 0:1],
                    scalar2=rstd[:, j:j + 1], op0=ALU.subtract, op1=ALU.mult)
                if general_ln:
                    nc.vector.tensor_mul(out=ht, in0=ht, in1=lnw_bc)
                    nc.vector.tensor_add(out=ht, in0=ht, in1=lnb_bc)
                for g in range(2):
                    pt = PST.tile([128, 512], BF16, tag="tr",
                                  name=f"pt_{i}_{g}")
                    for c in range(4):
                        cc = g * 4 + c
                        nc.tensor.transpose(
                            pt[:, c * 128:(c + 1) * 128],
                            ht[:, cc * 128:(cc + 1) * 128], ident)
                    for c in range(4):
                        cc = g * 4 + c
                        nc.vector.tensor_copy(
                            out=hT[:, cc, i * 128:(i + 1) * 128],
                            in_=pt[:, c * 128:(c + 1) * 128])
    hp_cm.__exit__(None, None, None)
    xp_cm.__exit__(None, None, None)
    dnp = ctx.enter_context(tc.tile_pool(name="dnp", bufs=8))
    lnp = ctx.enter_context(tc.tile_pool(name="lnp", bufs=8))
    recp = ctx.enter_context(tc.tile_pool(name="recp", bufs=8))

    # ---- Phase B+C+D interleaved: qkv, scores+exp, av ----
    qT = [qTp.tile([128, T], BF16, tag="qT", name=f"qT_{m}")
          for m in range(4)]
    kT = [kTp.tile([128, T], BF16, tag="kT", name=f"kT_{m}")
          for m in range(4)]
    # v2[tok, sc, h, 0:64] = v; col 64 = ones (softmax denominator)
    v2 = v2p.tile([128, NTB, HPC, 66], BF16, tag="v2")
    nc.vector.memset(v2[:, :, :, 64:65], 1.0)
    nc.vector.memset(v2[:, :, :, 65:66], 0.0)
    cat = [ctp.tile([128, T], BF16, tag="cat", name=f"cat_{m}")
           for m in range(4)]
    # per-head softmax denominator rows and their reciprocals
    dens = [dnp.tile([1, T], BF16, tag="den", name=f"den_{h}")
            for h in range(HPC)]
    lnds = [lnp.tile([1, T], F32, tag="lnd", name=f"lnd_{h}")
            for h in range(HPC)]
    recs = [recp.tile([1, T], BF16, tag="rec", name=f"rec_{h}")
            for h in range(HPC)]
    pav_sbs = [avp.tile([128, T], F32R, tag="avsb", name=f"avsb_{m}")
               for m in range(4)]

    # PSUM budget: PSQ 1x[128,512]=1 bank, PSS 3x[128,<=512]=3,
    # PSA 2x[66,1024]=4  -> 8 banks total.
    with tc.tile_pool(name="psq", bufs=1, space="PSUM") as PSQ, \
         tc.tile_pool(name="pss", bufs=3, space="PSUM") as PSS, \
         tc.tile_pool(name="psa", bufs=2, space="PSUM") as PSA:

        def qkv_m(m):
            for th in range(2):
                tsl = slice(th * 512, (th + 1) * 512)
                pq = PSQ.tile([128, 512], F32, tag="mm", name=f"pq_{m}_{th}")
                for cc in range(NCC):
                    nc.tensor.matmul(pq, wq_sb[:, cc, m * 128:(m + 1) * 128],
                                     hT[:, cc, tsl],
                                     start=(cc == 0), stop=(cc == NCC - 1))
                nc.vector.tensor_copy(out=qT[m][:, tsl], in_=pq)
                pk = PSQ.tile([128, 512], F32, tag="mm", name=f"pk_{m}_{th}")
                for cc in range(NCC):
                    nc.tensor.matmul(pk, wk_sb[:, cc, m * 128:(m + 1) * 128],
                                     hT[:, cc, tsl],
                                     start=(cc == 0), stop=(cc == NCC - 1))
                nc.vector.tensor_copy(out=kT[m][:, tsl], in_=pk)

        def v_tb(tb):
            pv = PSQ.tile([128, 512], F32, tag="mm", name=f"pv_{tb}")
            for cc in range(NCC):
                nc.tensor.matmul(pv, hT[:, cc, tb * 128:(tb + 1) * 128],
                                 wv_sb[:, cc, :],
                                 start=(cc == 0), stop=(cc == NCC - 1))
            nc.vector.tensor_copy(
                out=v2[:, tb, :, 0:64],
                in_=bass.AP(tensor=pv.tensor, offset=pv.offset,
                            ap=list(pv.ap[:1]) + [[64, HPC], [1, 64]]))

        def scores_h(h):
            """scoresT + exp for head h; returns e tiles per sc."""
            m, ho = h // 2, (h % 2) * 64
            qh = qT[m][ho:ho + 64, :]
            kh = kT[m][ho:ho + 64, :]
            es = []
            for sc in range(NTB):
                W = T - sc * 128
                e = ep.tile([128, W], BF16, tag=f"e{sc}", name=f"e_{h}_{sc}")
                for (o0, o1) in _bank_chunks(0, W):
                    ps = PSS.tile([128, o1 - o0], F32, tag="sc",
                                  name=f"ps_{h}_{sc}_{o0}")
                    nc.tensor.matmul(
                        ps,
                        qh[:, sc * 128:(sc + 1) * 128],
                        kh[:, sc * 128 + o0:sc * 128 + o1],
                        start=True, stop=True)
                    nc.scalar.activation(out=e[:, o0:o1], in_=ps,
                                         func=AF.Exp, scale=SCALE)
                # zero the upper triangle of the diagonal block (gpsimd,
                # off the PE->ACT critical chain)
                nc.gpsimd.affine_select(
                    out=e[:, 0:128], in_=e[:, 0:128], compare_op=ALU.is_ge,
                    fill=0.0, base=0, pattern=[[1, 128]],
                    channel_multiplier=-1)
                es.append(e)
            return es

        def av_h(h, es):
            """av for head h into pav_sb half + stash denominator row."""
            pav = PSA.tile([66, T], F32, tag="av", name=f"pav_{h}")
            for sc in range(NTB):
                lo = sc * 128
                for (o0, o1) in _bank_chunks(lo, T):
                    nc.tensor.matmul(
                        pav[:, o0:o1], v2[:, sc, h, :],
                        es[sc][:, o0 - lo:o1 - lo],
                        start=(sc == 0), stop=(sc == NTB - 1),
                        skip_group_check=True)
            ho = (h % 2) * 64
            nc.vector.tensor_copy(out=pav_sbs[h // 2][ho:ho + 64, :],
                                  in_=pav[0:64, :])
            nc.scalar.copy(out=dens[h], in_=pav[64:65, :])

        es0 = qkv_m(0) or scores_h(0)
        qkv_m(1)
        for tb in range(NTB):
            v_tb(tb)
        es1 = scores_h(1)
        qkv_m(2)
        es2 = scores_h(2)
        qkv_m(3)
        es3 = scores_h(3)

        es = {0: es0, 1: es1, 2: es2, 3: es3}
        for h in range(HPC):
            if h + 4 < HPC:
                es[h + 4] = scores_h(h + 4)
            av_h(h, es.pop(h))

    # ---- Phase E: 1/den via exp(-ln(den)), normalize, Wo projection ----
    with tc.tile_pool(name="psb", bufs=2, space="PSUM") as PSB, \
         tc.tile_pool(name="psp", bufs=2, space="PSUM") as PSP:
        with nc.allow_low_precision(reason="softmax denominator recip"):
            for grp in range(2):
                for i in range(4):
                    nc.scalar.activation(out=lnds[grp * 4 + i],
                                         in_=dens[grp * 4 + i], func=AF.Ln)
                for i in range(4):
                    nc.scalar.activation(out=recs[grp * 4 + i],
                                         in_=lnds[grp * 4 + i],
                                         func=AF.Exp, scale=-1.0)
        for m in range(4):
            # partition-broadcast each head's 1/den row via K=1 PE matmul
            prec = PSB.tile([128, T], F32, tag="prec", name=f"prec_{m}")
            for hh in range(2):
                for co in range(2):
                    nc.tensor.matmul(
                        prec[hh * 64:(hh + 1) * 64, co * 512:(co + 1) * 512],
                        ones64,
                        recs[2 * m + hh][:, co * 512:(co + 1) * 512],
                        start=True, stop=True)
            nc.vector.tensor_mul(out=cat[m], in0=pav_sbs[m], in1=prec)

        for tb in range(NTB):
            pp = PSP.tile([128, C], F32, tag="pp", name=f"pp_{tb}")
            for m in range(4):
                for co in range(2):
                    nc.tensor.matmul(
                        pp[:, co * 512:(co + 1) * 512],
                        cat[m][:, tb * 128:(tb + 1) * 128],
                        wo_sb[:, m, co * 512:(co + 1) * 512],
                        start=(m == 0), stop=(m == 3))
            o_sb = osp.tile([128, C], BF16, tag="o", name=f"o_{tb}")
            if tb % 2 == 0:
                nc.vector.tensor_copy(out=o_sb, in_=pp)
            else:
                nc.scalar.copy(out=o_sb, in_=pp)
            nc.sync.dma_start(out=pout[tb * 128:(tb + 1) * 128, :], in_=o_sb)


def _build_attn(general_ln: bool):
    nc = bacc.Bacc("TRN2", target_bir_lowering=False, debug=False)
    x = nc.dram_tensor("x", [T, C], BF16, kind="ExternalInput").ap()
    wq = nc.dram_tensor("wq", [128, NCC, 512], BF16, kind="ExternalInput").ap()
    wk = nc.dram_tensor("wk", [128, NCC, 512], BF16, kind="ExternalInput").ap()
    wv = nc.dram_tensor("wv", [128, NCC, 512], BF16, kind="ExternalInput").ap()
    wo = nc.dram_tensor("wo", [128, 4, C], BF16, kind="ExternalInput").ap()
    lnw = lnb = None
    if general_ln:
        lnw = nc.dram_tensor("lnw", [C], F32, kind="ExternalInput").ap()
        lnb = nc.dram_tensor("lnb", [C], F32, kind="ExternalInput").ap()
    ones_dram = nc.dram_tensor("ones", [1, 64], BF16,
                               kind="ExternalInput").ap()
    pout = nc.dram_tensor("pout", [T, C], BF16, kind="ExternalOutput").ap()
    with tile.TileContext(nc) as tc:
        with ExitStack() as ctx:
            _attn_body(ctx, tc, x, wq, wk, wv, wo, lnw, lnb, ones_dram, pout)
    nc.compile()
    return nc


# --------------------------------------------------------------------------
# kernel B: FFN, 512 rows per core
# --------------------------------------------------------------------------

def _ffn_body(ctx, tc, x2, x2l, w1, w2, b1, lnw, lnb, alpha, out):
    nc = tc.nc
    general_ln = lnw is not None

    const = ctx.enter_context(tc.tile_pool(name="const", bufs=1))
    xp = ctx.enter_context(tc.tile_pool(name="xp", bufs=NRB))
    hp = ctx.enter_context(tc.tile_pool(name="hp", bufs=5))
    hTp = ctx.enter_context(tc.tile_pool(name="hTp", bufs=1))
    stat = ctx.enter_context(tc.tile_pool(name="stat", bufs=2))
    ftp = ctx.enter_context(tc.tile_pool(name="ftp", bufs=1))
    tmp = ctx.enter_context(tc.tile_pool(name="tmp", bufs=2))
    xlp_cm = tc.tile_pool(name="xlp", bufs=NRB)
    xlp = xlp_cm.__enter__()
    # bf16 x2 tiles first: these DMAs gate the LN2 critical path; the
    # f32 residual tiles stream in behind the weights (needed ~100us in)
    xlts = []
    for r in range(NRB):
        xt = xlp.tile([128, C], BF16, tag="xl", name=f"xl_{r}")
        nc.sync.dma_start(out=xt, in_=x2l[r * 128:(r + 1) * 128, :])
        xlts.append(xt)
    scratch = const.tile([128, 128], F32)
    make_identity(nc, scratch)
    ident = const.tile([128, 128], BF16)
    nc.vector.tensor_copy(out=ident, in_=scratch)
    eps_t = const.tile([128, 1], F32)
    nc.vector.memset(eps_t, EPS)
    if general_ln:
        lnw_bc = const.tile([128, C], F32, tag="lnw")
        lnb_bc = const.tile([128, C], F32, tag="lnb")
        nc.sync.dma_start(
            out=lnw_bc,
            in_=bass.AP(tensor=lnw.tensor, offset=lnw.offset,
                        ap=[[0, 128]] + list(lnw.ap)))
        nc.sync.dma_start(
            out=lnb_bc,
            in_=bass.AP(tensor=lnb.tensor, offset=lnb.offset,
                        ap=[[0, 128]] + list(lnb.ap)))
    b1_sb = None
    if b1 is not None:
        b1_sb = const.tile([128, NHID], F32, tag="b1")
        nc.sync.dma_start(out=b1_sb, in_=b1.rearrange("(h p) -> p h", p=128))

    # weights: big resident tiles, streamed in chunks of 8 hidden blocks
    w1_sb = const.tile([128, NHID, NCC, 128], BF16, tag="w1")
    w2_sb = const.tile([128, NHID, C], BF16, tag="w2")
    for hg in range(4):
        hsl = slice(hg * 8, (hg + 1) * 8)
        nc.sync.dma_start(out=w1_sb[:, hsl, :, :], in_=w1[:, hsl, :, :])
    x2ts = []
    for r in range(NRB):
        xt = xp.tile([128, C], F32, tag="x", name=f"x_{r}")
        nc.sync.dma_start(out=xt, in_=x2[r * 128:(r + 1) * 128, :])
        x2ts.append(xt)
    for hg in range(4):
        hsl = slice(hg * 8, (hg + 1) * 8)
        nc.sync.dma_start(out=w2_sb[:, hsl, :], in_=w2[:, hsl, :])

    h2T = hTp.tile([128, NCC, RPC], BF16, tag="h2T")

    # ---- LN2 + transpose ----
    with tc.tile_pool(name="pst", bufs=2, space="PSUM") as PST:
        hts = []
        for grp in range(2):
            mvs = stat.tile([128, 2, 2], F32, tag="mvs", name=f"mvs_{grp}")
            rstd = stat.tile([128, 2], F32, tag="rstd", name=f"rs_{grp}")
            lnv = stat.tile([128, 2], F32, tag="lnv", name=f"lv_{grp}")
            for j in range(2):
                r = grp * 2 + j
                st = stat.tile([128, 2, 6], F32, tag="bn", name=f"bn_{r}")
                for k in range(2):
                    nc.vector.bn_stats(
                        out=st[:, k, :],
                        in_=xlts[r][:, k * 512:(k + 1) * 512])
                nc.vector.bn_aggr(out=mvs[:, j, :], in_=st)
            nc.scalar.activation(out=lnv, in_=mvs[:, :, 1], func=AF.Ln,
                                 bias=eps_t)
            nc.scalar.activation(out=rstd, in_=lnv, func=AF.Exp, scale=-0.5)
            for j in range(2):
                r = grp * 2 + j
                ht = hp.tile([128, C], BF16, tag="h", name=f"h_{r}")
                nc.vector.tensor_scalar(
                    out=ht, in0=xlts[r], scalar1=mvs[:, j, 0:1],
                    scalar2=rstd[:, j:j + 1], op0=ALU.subtract, op1=ALU.mult)
                if general_ln:
                    nc.vector.tensor_mul(out=ht, in0=ht, in1=lnw_bc)
                    nc.vector.tensor_add(out=ht, in0=ht, in1=lnb_bc)
                hts.append(ht)
        for cc in range(NCC):
            pt = PST.tile([128, RPC], BF16, tag="tr", name=f"pt_{cc}")
            for r in range(NRB):
                nc.tensor.transpose(
                    pt[:, r * 128:(r + 1) * 128],
                    hts[r][:, cc * 128:(cc + 1) * 128], ident)
            nc.vector.tensor_copy(out=h2T[:, cc, :], in_=pt)
    xlp_cm.__exit__(None, None, None)
    osp = ctx.enter_context(tc.tile_pool(name="osp", bufs=2))

    # ---- W1 + PReLU ----
    fbig = ftp.tile([128, NHID, RPC], BF16, tag="ft")
    with tc.tile_pool(name="psf", bufs=2, space="PSUM") as PSF:
        for h in range(NHID):
            pf = PSF.tile([128, RPC], F32, tag="f", name=f"pf_{h}")
            for cc in range(NCC):
                nc.tensor.matmul(pf, w1_sb[:, h, cc, :], h2T[:, cc, :],
                                 start=(cc == 0), stop=(cc == NCC - 1))
            if b1_sb is not None:
                nc.vector.tensor_scalar_add(out=pf, in0=pf,
                                            scalar1=b1_sb[:, h:h + 1])
            t1 = tmp.tile([128, RPC], F32, tag="t1", name=f"t1_{h}")
            nc.vector.tensor_scalar(
                out=t1, in0=pf, scalar1=0.0, scalar2=alpha - 1.0,
                op0=ALU.min, op1=ALU.mult)
            nc.vector.tensor_add(out=fbig[:, h, :], in0=pf, in1=t1)

    # ---- W2 + residual ----
    with tc.tile_pool(name="pso", bufs=2, space="PSUM") as PSO:
        for tb in range(NRB):
            po = PSO.tile([128, C], F32, tag="o", name=f"po_{tb}")
            for h in range(NHID):
                for co in range(2):
                    nc.tensor.matmul(
                        po[:, co * 512:(co + 1) * 512],
                        fbig[:, h, tb * 128:(tb + 1) * 128],
                        w2_sb[:, h, co * 512:(co + 1) * 512],
                        start=(h == 0), stop=(h == NHID - 1))
            o_sb = osp.tile([128, C], F32, tag="osb", name=f"osb_{tb}")
            nc.vector.tensor_add(out=o_sb, in0=po, in1=x2ts[tb])
            nc.sync.dma_start(out=out[tb * 128:(tb + 1) * 128, :], in_=o_sb)


def _build_ffn(general_ln: bool, has_b1: bool, alpha: float):
    nc = bacc.Bacc("TRN2", target_bir_lowering=False, debug=False)
    x2 = nc.dram_tensor("x2", [RPC, C], F32, kind="ExternalInput").ap()
    x2l = nc.dram_tensor("x2l", [RPC, C], BF16, kind="ExternalInput").ap()
    w1 = nc.dram_tensor("w1", [128, NHID, NCC, 128], BF16,
                        kind="ExternalInput").ap()
    w2 = nc.dram_tensor("w2", [128, NHID, C], BF16,
                        kind="ExternalInput").ap()
    b1 = lnw = lnb = None
    if has_b1:
        b1 = nc.dram_tensor("b1", [4 * C], F32, kind="ExternalInput").ap()
    if general_ln:
        lnw = nc.dram_tensor("lnw", [C], F32, kind="ExternalInput").ap()
        lnb = nc.dram_tensor("lnb", [C], F32, kind="ExternalInput").ap()
    out = nc.dram_tensor("out", [RPC, C], F32, kind="ExternalOutput").ap()
    with tile.TileContext(nc) as tc:
        with ExitStack() as ctx:
            _ffn_body(ctx, tc, x2, x2l, w1, w2, b1, lnw, lnb, alpha, out)
    nc.compile()
    return nc


# --------------------------------------------------------------------------
# host orchestration
# --------------------------------------------------------------------------

_NC_CACHE = {}

# bench-only instrumentation: when KBENCH_TRACE is set, launches run with
# trace=True and per-launch device exec_time_ns is appended here.
_TRACE = bool(os.environ.get("KBENCH_TRACE"))
EXEC_NS = []
TRACE_PATHS = []


def _run_spmd(nc, in_maps):
    res = run_bass_kernel_spmd(nc, in_maps, list(range(NCORES)),
                               trace=_TRACE,
                               trace_cores=list(range(NCORES)) if _TRACE
                               else None)
    if _TRACE:
        EXEC_NS.append(res.exec_time_ns)
        if res.instructions_and_trace is not None:
            TRACE_PATHS.append(res.instructions_and_trace[1])
    return res


def _bf16(a):
    import ml_dtypes
    return np.ascontiguousarray(np.asarray(a, np.float32)
                                .astype(ml_dtypes.bfloat16))


def _get_attn_nc(general_ln):
    key = ("attn", general_ln)
    if key not in _NC_CACHE:
        _NC_CACHE[key] = _build_attn(general_ln)
    return _NC_CACHE[key]


def _get_ffn_nc(general_ln, has_b1, alpha):
    key = ("ffn", general_ln, has_b1, float(alpha))
    if key not in _NC_CACHE:
        _NC_CACHE[key] = _build_ffn(general_ln, has_b1, float(alpha))
    return _NC_CACHE[key]


def _attn_weights(Wq, Wk, Wv, Wo):
    """Per-core weight arrays in the device layouts."""
    per_core = []
    for c in range(NCORES):
        hh = c % 2
        h0 = HPC * hh
        # [C, 512] -> [128, NCC, 512]
        def wlay(Wx):
            catw = np.concatenate([Wx[h] for h in range(h0, h0 + HPC)],
                                  axis=1)  # [C, 512]
            return _bf16(catw.reshape(NCC, 128, 512).transpose(1, 0, 2))
        wo = _bf16(Wo[hh * 512:(hh + 1) * 512].reshape(4, 128, C)
                   .transpose(1, 0, 2))
        per_core.append((wlay(Wq), wlay(Wk), wlay(Wv), wo))
    return per_core


def run_attn(x_flat, Wq, Wk, Wv, Wo, ln1_w, ln1_b):
    """Returns list of per-core partial projections [T, C] f32."""
    trivial = bool(np.all(ln1_w == 1.0) and np.all(ln1_b == 0.0))
    nc = _get_attn_nc(not trivial)
    wts = _attn_weights(Wq, Wk, Wv, Wo)
    in_maps = []
    for c in range(NCORES):
        b = c // 2
        wq, wk, wv, wo = wts[c]
        import ml_dtypes
        m = {"x": _bf16(x_flat[b * T:(b + 1) * T]),
             "wq": wq, "wk": wk, "wv": wv, "wo": wo,
             "ones": np.ones((1, 64), ml_dtypes.bfloat16)}
        if not trivial:
            m["lnw"] = np.asarray(ln1_w, np.float32)
            m["lnb"] = np.asarray(ln1_b, np.float32)
        in_maps.append(m)
    res = _run_spmd(nc, in_maps)
    return [res.results[c]["pout"] for c in range(NCORES)]


def run_ffn(x2_flat, W1, b1, W2, ln2_w, ln2_b, alpha):
    trivial = bool(np.all(ln2_w == 1.0) and np.all(ln2_b == 0.0))
    has_b1 = bool(np.any(b1 != 0.0))
    nc = _get_ffn_nc(not trivial, has_b1, alpha)
    w1l = _bf16(np.asarray(W1, np.float32)
                .reshape(NCC, 128, NHID, 128).transpose(1, 2, 0, 3))
    w2l = _bf16(np.asarray(W2, np.float32)
                .reshape(NHID, 128, C).transpose(1, 0, 2))
    in_maps = []
    for c in range(NCORES):
        rows = np.ascontiguousarray(x2_flat[RPC * c:RPC * (c + 1)])
        m = {"x2": rows, "x2l": _bf16(rows), "w1": w1l, "w2": w2l}
        if has_b1:
            m["b1"] = np.asarray(b1, np.float32)
        if not trivial:
            m["lnw"] = np.asarray(ln2_w, np.float32)
            m["lnb"] = np.asarray(ln2_b, np.float32)
        in_maps.append(m)
    res = _run_spmd(nc, in_maps)
    return np.concatenate(
        [res.results[c]["out"] for c in range(NCORES)], axis=0)


def kernel(x, ln1_w, ln1_b, Wk, Wq, Wv, Wo, bo, ln2_w, ln2_b, W1, b1,
           prelu_a, W2, b2):
    x = np.asarray(x, np.float32)
    x_flat = np.ascontiguousarray(x.reshape(B * T, C))
    alpha = float(np.asarray(prelu_a))

    parts = run_attn(x_flat, np.asarray(Wq, np.float32),
                     np.asarray(Wk, np.float32),
                     np.asarray(Wv, np.float32),
                     np.asarray(Wo, np.float32),
                     np.asarray(ln1_w, np.float32),
                     np.asarray(ln1_b, np.float32))
    # host reduction: x2 = x + partial_even + partial_odd (+ bo)
    x2 = np.empty_like(x_flat)
    for b in range(B):
        x2[b * T:(b + 1) * T] = (x_flat[b * T:(b + 1) * T]
                                 + parts[2 * b].astype(np.float32)
                                 + parts[2 * b + 1].astype(np.float32))
    bo = np.asarray(bo, np.float32)
    if np.any(bo != 0.0):
        x2 += bo
    out = run_ffn(x2, W1, np.asarray(b1, np.float32), W2,
                  np.asarray(ln2_w, np.float32),
                  np.asarray(ln2_b, np.float32), alpha)
    b2 = np.asarray(b2, np.float32)
    if np.any(b2 != 0.0):
        out = out + b2
    return out.reshape(B, T, C).astype(np.float32)


# revision 44
# speedup vs baseline: 1.1244x; 1.0210x over previous
"""Trainium2 Bass kernel for a dense pre-LN transformer block.

B=4, T=1024, C=1024, H=16 heads (head_size 64).

Distribution over 8 NeuronCores, two SPMD launches with a free host-side
reduction between them:

  Launch A (attention): core c works on batch b=c//2 and head-half
  hh=c%2 (8 heads). It computes LN1 for its batch only, projects
  q/k/v for its heads, runs causal softmax(k@q^T)-attention in the
  transposed-scores layout, and multiplies by its slice of Wo rows,
  producing a PARTIAL projection [T, C] (f32) for its batch.

  Host: x2[b] = x[b] + part[2b] + part[2b+1] (+bo).

  Launch B (FFN): core c runs LN2 + W1/PReLU/W2 + residual on rows
  [512c, 512(c+1)) of x2.

Matmul dtype strategy: the PE cost depends only on the MOVING operand
dtype and its free size (1 cycle/row for bf16 at any N, f32r at N>=256).
Activations that move (hT, wo, w2 stream, e) stay f32r/bf16 chosen for
SBUF fit; weights that sit stationary are bf16 (0.4% quantization).
Accumulation is always f32 in PSUM.
"""

import os
from contextlib import ExitStack

import numpy as np

import concourse.bass as bass
import concourse.tile as tile
from concourse import bacc, mybir
from concourse.bass_utils import run_bass_kernel_spmd
from concourse.masks import make_identity

F32 = mybir.dt.float32
F32R = mybir.dt.float32r
BF16 = mybir.dt.bfloat16
FP8 = mybir.dt.float8e4
AF = mybir.ActivationFunctionType
ALU = mybir.AluOpType

B, T, C, H, HS = 4, 1024, 1024, 16, 64
NCORES = 8
EPS = 1e-5
SCALE = float(C) ** -0.5  # folded into the softmax exp
NEG = -1e30

NTB = T // 128   # 8 token blocks per batch
NCC = C // 128   # 8 channel chunks
HPC = H // 2     # 8 heads per core
RPC = (B * T) // NCORES  # 512 rows per core in launch B
NRB = RPC // 128         # 4 row blocks
NHID = 4 * C // 128      # 32 hidden chunks


def _bank_chunks(lo, hi):
    """Split [lo, hi) at 512-column PSUM bank boundaries."""
    out = []
    o = lo
    while o < hi:
        n = min(512 - (o % 512), hi - o)
        out.append((o, o + n))
        o += n
    return out


# --------------------------------------------------------------------------
# kernel A: attention, one batch + 8 heads per core
# --------------------------------------------------------------------------

def _attn_body(ctx, tc, x, wq, wk, wv, wo, lnw, lnb, ones_dram, pout):
    nc = tc.nc
    general_ln = lnw is not None

    const = ctx.enter_context(tc.tile_pool(name="const", bufs=1))
    hTp = ctx.enter_context(tc.tile_pool(name="hTp", bufs=1))
    qTp = ctx.enter_context(tc.tile_pool(name="qTp", bufs=4))
    kTp = ctx.enter_context(tc.tile_pool(name="kTp", bufs=4))
    v2p = ctx.enter_context(tc.tile_pool(name="v2p", bufs=1))
    stat = ctx.enter_context(tc.tile_pool(name="stat", bufs=2))
    ep = ctx.enter_context(tc.tile_pool(name="ep", bufs=3))
    avp = ctx.enter_context(tc.tile_pool(name="avp", bufs=4))
    ctp = ctx.enter_context(tc.tile_pool(name="ctp", bufs=4))
    osp = ctx.enter_context(tc.tile_pool(name="osp", bufs=2))

    xp_cm = tc.tile_pool(name="xp", bufs=8)
    hp_cm = tc.tile_pool(name="hp", bufs=3)
    xp = xp_cm.__enter__()
    hp = hp_cm.__enter__()
    # x tiles first: these DMAs gate the LN1 critical path
    xts = []
    for i in range(NTB):
        xt = xp.tile([128, C], BF16, tag="x", name=f"x_{i}")
        nc.sync.dma_start(out=xt, in_=x[i * 128:(i + 1) * 128, :])
        xts.append(xt)

    scratch = const.tile([128, 128], F32)
    make_identity(nc, scratch)
    ident = const.tile([128, 128], BF16)
    nc.vector.tensor_copy(out=ident, in_=scratch)
    eps_t = const.tile([128, 1], F32)
    nc.vector.memset(eps_t, EPS)
    ones64 = const.tile([1, 64], BF16)
    nc.sync.dma_start(out=ones64, in_=ones_dram)
    if general_ln:
        lnw_bc = const.tile([128, C], F32, tag="lnw")
        lnb_bc = const.tile([128, C], F32, tag="lnb")
        nc.sync.dma_start(
            out=lnw_bc,
            in_=bass.AP(tensor=lnw.tensor, offset=lnw.offset,
                        ap=[[0, 128]] + list(lnw.ap)))
        nc.sync.dma_start(
            out=lnb_bc,
            in_=bass.AP(tensor=lnb.tensor, offset=lnb.offset,
                        ap=[[0, 128]] + list(lnb.ap)))

    # weights resident in SBUF (after x: LN1 must not wait behind these)
    wq_sb = const.tile([128, NCC, 512], BF16, tag="wq")
    wk_sb = const.tile([128, NCC, 512], BF16, tag="wk")
    wv_sb = const.tile([128, NCC, 512], BF16, tag="wv")
    wo_sb = const.tile([128, 4, C], BF16, tag="wo")
    nc.sync.dma_start(out=wq_sb, in_=wq)
    nc.sync.dma_start(out=wk_sb, in_=wk)
    nc.sync.dma_start(out=wv_sb, in_=wv)
    nc.sync.dma_start(out=wo_sb, in_=wo)

    hT = hTp.tile([128, NCC, T], BF16, tag="hT")

    # ---- Phase A: LN1 (own batch only) + transpose, 4 groups of 2 ----
    with tc.tile_pool(name="pst", bufs=2, space="PSUM") as PST:
        for grp in range(4):
            mvs = stat.tile([128, 2, 2], F32, tag="mvs", name=f"mvs_{grp}")
            rstd = stat.tile([128, 2], F32, tag="rstd", name=f"rstd_{grp}")
            lnv = stat.tile([128, 2], F32, tag="lnv", name=f"lnv_{grp}")
            for j in range(2):
                i = grp * 2 + j
                st = stat.tile([128, 2, 6], F32, tag="bn", name=f"bn_{i}")
                for k in range(2):
                    nc.vector.bn_stats(out=st[:, k, :],
                                       in_=xts[i][:, k * 512:(k + 1) * 512])
                nc.vector.bn_aggr(out=mvs[:, j, :], in_=st)
            nc.scalar.activation(out=lnv, in_=mvs[:, :, 1], func=AF.Ln,
                                 bias=eps_t)
            nc.scalar.activation(out=rstd, in_=lnv, func=AF.Exp, scale=-0.5)
            for j in range(2):
                i = grp * 2 + j
                ht = hp.tile([128, C], BF16, tag="h", name=f"h_{i}")
                nc.vector.tensor_scalar(
                    out=ht, in0=xts[i], scalar1=mvs[:, j, 0:1],
                    scalar2=rstd[:, j:j + 1], op0=ALU.subtract, op1=ALU.mult)
                if general_ln:
                    nc.vector.tensor_mul(out=ht, in0=ht, in1=lnw_bc)
                    nc.vector.tensor_add(out=ht, in0=ht, in1=lnb_bc)
                for g in range(2):
                    pt = PST.tile([128, 512], BF16, tag="tr",
                                  name=f"pt_{i}_{g}")
                    for c in range(4):
                        cc = g * 4 + c
                        nc.tensor.transpose(
                            pt[:, c * 128:(c + 1) * 128],
                            ht[:, cc * 128:(cc + 1) * 128], ident)
                    for c in range(4):
                        cc = g * 4 + c
                        nc.vector.tensor_copy(
                            out=hT[:, cc, i * 128:(i + 1) * 128],
                            in_=pt[:, c * 128:(c + 1) * 128])
    hp_cm.__exit__(None, None, None)
    xp_cm.__exit__(None, None, None)
    lnp = ctx.enter_context(tc.tile_pool(name="lnp", bufs=8))
    recp = ctx.enter_context(tc.tile_pool(name="recp", bufs=8))

    # ---- Phase B+C+D interleaved: qkv, scores+exp, av ----
    qT = [qTp.tile([128, T], BF16, tag="qT", name=f"qT_{m}")
          for m in range(4)]
    kT = [kTp.tile([128, T], BF16, tag="kT", name=f"kT_{m}")
          for m in range(4)]
    # v2[tok, sc, h, 0:64] = v; col 64 = ones (softmax denominator)
    v2 = v2p.tile([128, NTB, HPC, 66], BF16, tag="v2")
    nc.vector.memset(v2[:, :, :, 64:65], 1.0)
    nc.vector.memset(v2[:, :, :, 65:66], 0.0)
    cat = [ctp.tile([128, T], BF16, tag="cat", name=f"cat_{m}")
           for m in range(4)]
    # per-head ln(softmax denominator) rows and their reciprocals
    lnds = [lnp.tile([1, T], F32, tag="lnd", name=f"lnd_{h}")
            for h in range(HPC)]
    recs = [recp.tile([1, T], BF16, tag="rec", name=f"rec_{h}")
            for h in range(HPC)]
    pav_sbs = [avp.tile([128, T], F32R, tag="avsb", name=f"avsb_{m}")
               for m in range(4)]

    # PSUM budget: PSQ 1x[128,512]=1 bank, PSS 3x[128,<=512]=3,
    # PSA 2x[66,1024]=4  -> 8 banks total.
    with tc.tile_pool(name="psq", bufs=1, space="PSUM") as PSQ, \
         tc.tile_pool(name="pss", bufs=3, space="PSUM") as PSS, \
         tc.tile_pool(name="psa", bufs=2, space="PSUM") as PSA:

        def qkv_m(m):
            for th in range(2):
                tsl = slice(th * 512, (th + 1) * 512)
                pq = PSQ.tile([128, 512], F32, tag="mm", name=f"pq_{m}_{th}")
                for cc in range(NCC):
                    nc.tensor.matmul(pq, wq_sb[:, cc, m * 128:(m + 1) * 128],
                                     hT[:, cc, tsl],
                                     start=(cc == 0), stop=(cc == NCC - 1))
                nc.vector.tensor_copy(out=qT[m][:, tsl], in_=pq)
                pk = PSQ.tile([128, 512], F32, tag="mm", name=f"pk_{m}_{th}")
                for cc in range(NCC):
                    nc.tensor.matmul(pk, wk_sb[:, cc, m * 128:(m + 1) * 128],
                                     hT[:, cc, tsl],
                                     start=(cc == 0), stop=(cc == NCC - 1))
                nc.vector.tensor_copy(out=kT[m][:, tsl], in_=pk)

        def v_tb(tb):
            pv = PSQ.tile([128, 512], F32, tag="mm", name=f"pv_{tb}")
            for cc in range(NCC):
                nc.tensor.matmul(pv, hT[:, cc, tb * 128:(tb + 1) * 128],
                                 wv_sb[:, cc, :],
                                 start=(cc == 0), stop=(cc == NCC - 1))
            nc.vector.tensor_copy(
                out=v2[:, tb, :, 0:64],
                in_=bass.AP(tensor=pv.tensor, offset=pv.offset,
                            ap=list(pv.ap[:1]) + [[64, HPC], [1, 64]]))

        def scores_h(h):
            """scoresT + exp for head h; returns e tiles per sc."""
            m, ho = h // 2, (h % 2) * 64
            qh = qT[m][ho:ho + 64, :]
            kh = kT[m][ho:ho + 64, :]
            es = []
            for sc in range(NTB):
                W = T - sc * 128
                e = ep.tile([128, W], BF16, tag=f"e{sc}", name=f"e_{h}_{sc}")
                for (o0, o1) in _bank_chunks(0, W):
                    ps = PSS.tile([128, o1 - o0], F32, tag="sc",
                                  name=f"ps_{h}_{sc}_{o0}")
                    nc.tensor.matmul(
                        ps,
                        qh[:, sc * 128:(sc + 1) * 128],
                        kh[:, sc * 128 + o0:sc * 128 + o1],
                        start=True, stop=True)
                    nc.scalar.activation(out=e[:, o0:o1], in_=ps,
                                         func=AF.Exp, scale=SCALE)
                # zero the upper triangle of the diagonal block (gpsimd,
                # off the PE->ACT critical chain)
                nc.gpsimd.affine_select(
                    out=e[:, 0:128], in_=e[:, 0:128], compare_op=ALU.is_ge,
                    fill=0.0, base=0, pattern=[[1, 128]],
                    channel_multiplier=-1)
                es.append(e)
            return es

        def av_h(h, es):
            """av for head h into pav_sb half + stash denominator row."""
            pav = PSA.tile([66, T], F32, tag="av", name=f"pav_{h}")
            for sc in range(NTB):
                lo = sc * 128
                for (o0, o1) in _bank_chunks(lo, T):
                    nc.tensor.matmul(
                        pav[:, o0:o1], v2[:, sc, h, :],
                        es[sc][:, o0 - lo:o1 - lo],
                        start=(sc == 0), stop=(sc == NTB - 1),
                        skip_group_check=True)
            ho = (h % 2) * 64
            nc.vector.tensor_copy(out=pav_sbs[h // 2][ho:ho + 64, :],
                                  in_=pav[0:64, :])
            nc.scalar.activation(out=lnds[h], in_=pav[64:65, :],
                                 func=AF.Ln)

        es0 = qkv_m(0) or scores_h(0)
        qkv_m(1)
        for tb in range(NTB):
            v_tb(tb)
        es1 = scores_h(1)
        qkv_m(2)
        es2 = scores_h(2)
        qkv_m(3)
        es3 = scores_h(3)

        es = {0: es0, 1: es1, 2: es2, 3: es3}
        for h in range(HPC):
            if h + 4 < HPC:
                es[h + 4] = scores_h(h + 4)
            av_h(h, es.pop(h))

    # ---- Phase E: 1/den via exp(-ln(den)), normalize, Wo projection ----
    with tc.tile_pool(name="psb", bufs=2, space="PSUM") as PSB, \
         tc.tile_pool(name="psp", bufs=2, space="PSUM") as PSP:
        with nc.allow_low_precision(reason="softmax denominator recip"):
            for h in range(HPC):
                nc.scalar.activation(out=recs[h], in_=lnds[h],
                                     func=AF.Exp, scale=-1.0)
        for m in range(4):
            # partition-broadcast each head's 1/den row via K=1 PE matmul
            prec = PSB.tile([128, T], F32, tag="prec", name=f"prec_{m}")
            for hh in range(2):
                for co in range(2):
                    nc.tensor.matmul(
                        prec[hh * 64:(hh + 1) * 64, co * 512:(co + 1) * 512],
                        ones64,
                        recs[2 * m + hh][:, co * 512:(co + 1) * 512],
                        start=True, stop=True)
            nc.vector.tensor_mul(out=cat[m], in0=pav_sbs[m], in1=prec)

        for tb in range(NTB):
            pp = PSP.tile([128, C], F32, tag="pp", name=f"pp_{tb}")
            for m in range(4):
                for co in range(2):
                    nc.tensor.matmul(
                        pp[:, co * 512:(co + 1) * 512],
                        cat[m][:, tb * 128:(tb + 1) * 128],
                        wo_sb[:, m, co * 512:(co + 1) * 512],
                        start=(m == 0), stop=(m == 3))
            o_sb = osp.tile([128, C], BF16, tag="o", name=f"o_{tb}")
            if tb % 2 == 0:
                nc.vector.tensor_copy(out=o_sb, in_=pp)
            else:
                nc.scalar.copy(out=o_sb, in_=pp)
            nc.sync.dma_start(out=pout[tb * 128:(tb + 1) * 128, :], in_=o_sb)


def _build_attn(general_ln: bool):
    nc = bacc.Bacc("TRN2", target_bir_lowering=False, debug=False)
    x = nc.dram_tensor("x", [T, C], BF16, kind="ExternalInput").ap()
    wq = nc.dram_tensor("wq", [128, NCC, 512], BF16, kind="ExternalInput").ap()
    wk = nc.dram_tensor("wk", [128, NCC, 512], BF16, kind="ExternalInput").ap()
    wv = nc.dram_tensor("wv", [128, NCC, 512], BF16, kind="ExternalInput").ap()
    wo = nc.dram_tensor("wo", [128, 4, C], BF16, kind="ExternalInput").ap()
    lnw = lnb = None
    if general_ln:
        lnw = nc.dram_tensor("lnw", [C], F32, kind="ExternalInput").ap()
        lnb = nc.dram_tensor("lnb", [C], F32, kind="ExternalInput").ap()
    ones_dram = nc.dram_tensor("ones", [1, 64], BF16,
                               kind="ExternalInput").ap()
    pout = nc.dram_tensor("pout", [T, C], BF16, kind="ExternalOutput").ap()
    with tile.TileContext(nc) as tc:
        with ExitStack() as ctx:
            _attn_body(ctx, tc, x, wq, wk, wv, wo, lnw, lnb, ones_dram, pout)
    nc.compile()
    return nc


# --------------------------------------------------------------------------
# kernel B: FFN, 512 rows per core
# --------------------------------------------------------------------------

def _ffn_body(ctx, tc, x2, x2l, w1, w2, b1, lnw, lnb, alpha, out):
    nc = tc.nc
    general_ln = lnw is not None

    const = ctx.enter_context(tc.tile_pool(name="const", bufs=1))
    xp = ctx.enter_context(tc.tile_pool(name="xp", bufs=NRB))
    hp = ctx.enter_context(tc.tile_pool(name="hp", bufs=5))
    hTp = ctx.enter_context(tc.tile_pool(name="hTp", bufs=1))
    stat = ctx.enter_context(tc.tile_pool(name="stat", bufs=2))
    ftp = ctx.enter_context(tc.tile_pool(name="ftp", bufs=1))
    tmp = ctx.enter_context(tc.tile_pool(name="tmp", bufs=2))
    xlp_cm = tc.tile_pool(name="xlp", bufs=NRB)
    xlp = xlp_cm.__enter__()
    # bf16 x2 tiles first: these DMAs gate the LN2 critical path; the
    # f32 residual tiles stream in behind the weights (needed ~100us in)
    xlts = []
    for r in range(NRB):
        xt = xlp.tile([128, C], BF16, tag="xl", name=f"xl_{r}")
        nc.sync.dma_start(out=xt, in_=x2l[r * 128:(r + 1) * 128, :])
        xlts.append(xt)
    scratch = const.tile([128, 128], F32)
    make_identity(nc, scratch)
    ident = const.tile([128, 128], BF16)
    nc.vector.tensor_copy(out=ident, in_=scratch)
    eps_t = const.tile([128, 1], F32)
    nc.vector.memset(eps_t, EPS)
    if general_ln:
        lnw_bc = const.tile([128, C], F32, tag="lnw")
        lnb_bc = const.tile([128, C], F32, tag="lnb")
        nc.sync.dma_start(
            out=lnw_bc,
            in_=bass.AP(tensor=lnw.tensor, offset=lnw.offset,
                        ap=[[0, 128]] + list(lnw.ap)))
        nc.sync.dma_start(
            out=lnb_bc,
            in_=bass.AP(tensor=lnb.tensor, offset=lnb.offset,
                        ap=[[0, 128]] + list(lnb.ap)))
    b1_sb = None
    if b1 is not None:
        b1_sb = const.tile([128, NHID], F32, tag="b1")
        nc.sync.dma_start(out=b1_sb, in_=b1.rearrange("(h p) -> p h", p=128))

    # weights: big resident tiles, streamed in chunks of 8 hidden blocks
    w1_sb = const.tile([128, NHID, NCC, 128], BF16, tag="w1")
    w2_sb = const.tile([128, NHID, C], BF16, tag="w2")
    for hg in range(4):
        hsl = slice(hg * 8, (hg + 1) * 8)
        nc.sync.dma_start(out=w1_sb[:, hsl, :, :], in_=w1[:, hsl, :, :])
    x2ts = []
    for r in range(NRB):
        xt = xp.tile([128, C], F32, tag="x", name=f"x_{r}")
        nc.sync.dma_start(out=xt, in_=x2[r * 128:(r + 1) * 128, :])
        x2ts.append(xt)
    for hg in range(4):
        hsl = slice(hg * 8, (hg + 1) * 8)
        nc.sync.dma_start(out=w2_sb[:, hsl, :], in_=w2[:, hsl, :])

    h2T = hTp.tile([128, NCC, RPC], BF16, tag="h2T")

    # ---- LN2 + transpose ----
    with tc.tile_pool(name="pst", bufs=2, space="PSUM") as PST:
        hts = []
        for grp in range(2):
            mvs = stat.tile([128, 2, 2], F32, tag="mvs", name=f"mvs_{grp}")
            rstd = stat.tile([128, 2], F32, tag="rstd", name=f"rs_{grp}")
            lnv = stat.tile([128, 2], F32, tag="lnv", name=f"lv_{grp}")
            for j in range(2):
                r = grp * 2 + j
                st = stat.tile([128, 2, 6], F32, tag="bn", name=f"bn_{r}")
                for k in range(2):
                    nc.vector.bn_stats(
                        out=st[:, k, :],
                        in_=xlts[r][:, k * 512:(k + 1) * 512])
                nc.vector.bn_aggr(out=mvs[:, j, :], in_=st)
            nc.scalar.activation(out=lnv, in_=mvs[:, :, 1], func=AF.Ln,
                                 bias=eps_t)
            nc.scalar.activation(out=rstd, in_=lnv, func=AF.Exp, scale=-0.5)
            for j in range(2):
                r = grp * 2 + j
                ht = hp.tile([128, C], BF16, tag="h", name=f"h_{r}")
                nc.vector.tensor_scalar(
                    out=ht, in0=xlts[r], scalar1=mvs[:, j, 0:1],
                    scalar2=rstd[:, j:j + 1], op0=ALU.subtract, op1=ALU.mult)
                if general_ln:
                    nc.vector.tensor_mul(out=ht, in0=ht, in1=lnw_bc)
                    nc.vector.tensor_add(out=ht, in0=ht, in1=lnb_bc)
                hts.append(ht)
        for cc in range(NCC):
            pt = PST.tile([128, RPC], BF16, tag="tr", name=f"pt_{cc}")
            for r in range(NRB):
                nc.tensor.transpose(
                    pt[:, r * 128:(r + 1) * 128],
                    hts[r][:, cc * 128:(cc + 1) * 128], ident)
            nc.vector.tensor_copy(out=h2T[:, cc, :], in_=pt)
    xlp_cm.__exit__(None, None, None)
    osp = ctx.enter_context(tc.tile_pool(name="osp", bufs=2))

    # ---- W1 + PReLU ----
    fbig = ftp.tile([128, NHID, RPC], BF16, tag="ft")
    with tc.tile_pool(name="psf", bufs=2, space="PSUM") as PSF:
        for h in range(NHID):
            pf = PSF.tile([128, RPC], F32, tag="f", name=f"pf_{h}")
            for cc in range(NCC):
                nc.tensor.matmul(pf, w1_sb[:, h, cc, :], h2T[:, cc, :],
                                 start=(cc == 0), stop=(cc == NCC - 1))
            if b1_sb is not None:
                nc.vector.tensor_scalar_add(out=pf, in0=pf,
                                            scalar1=b1_sb[:, h:h + 1])
            t1 = tmp.tile([128, RPC], F32, tag="t1", name=f"t1_{h}")
            nc.vector.tensor_scalar(
                out=t1, in0=pf, scalar1=0.0, scalar2=alpha - 1.0,
                op0=ALU.min, op1=ALU.mult)
            nc.vector.tensor_add(out=fbig[:, h, :], in0=pf, in1=t1)

    # ---- W2 + residual ----
    with tc.tile_pool(name="pso", bufs=2, space="PSUM") as PSO:
        for tb in range(NRB):
            po = PSO.tile([128, C], F32, tag="o", name=f"po_{tb}")
            for h in range(NHID):
                for co in range(2):
                    nc.tensor.matmul(
                        po[:, co * 512:(co + 1) * 512],
                        fbig[:, h, tb * 128:(tb + 1) * 128],
                        w2_sb[:, h, co * 512:(co + 1) * 512],
                        start=(h == 0), stop=(h == NHID - 1))
            o_sb = osp.tile([128, C], F32, tag="osb", name=f"osb_{tb}")
            nc.vector.tensor_add(out=o_sb, in0=po, in1=x2ts[tb])
            nc.sync.dma_start(out=out[tb * 128:(tb + 1) * 128, :], in_=o_sb)


def _build_ffn(general_ln: bool, has_b1: bool, alpha: float):
    nc = bacc.Bacc("TRN2", target_bir_lowering=False, debug=False)
    x2 = nc.dram_tensor("x2", [RPC, C], F32, kind="ExternalInput").ap()
    x2l = nc.dram_tensor("x2l", [RPC, C], BF16, kind="ExternalInput").ap()
    w1 = nc.dram_tensor("w1", [128, NHID, NCC, 128], BF16,
                        kind="ExternalInput").ap()
    w2 = nc.dram_tensor("w2", [128, NHID, C], BF16,
                        kind="ExternalInput").ap()
    b1 = lnw = lnb = None
    if has_b1:
        b1 = nc.dram_tensor("b1", [4 * C], F32, kind="ExternalInput").ap()
    if general_ln:
        lnw = nc.dram_tensor("lnw", [C], F32, kind="ExternalInput").ap()
        lnb = nc.dram_tensor("lnb", [C], F32, kind="ExternalInput").ap()
    out = nc.dram_tensor("out", [RPC, C], F32, kind="ExternalOutput").ap()
    with tile.TileContext(nc) as tc:
        with ExitStack() as ctx:
            _ffn_body(ctx, tc, x2, x2l, w1, w2, b1, lnw, lnb, alpha, out)
    nc.compile()
    return nc


# --------------------------------------------------------------------------
# host orchestration
# --------------------------------------------------------------------------

_NC_CACHE = {}

# bench-only instrumentation: when KBENCH_TRACE is set, launches run with
# trace=True and per-launch device exec_time_ns is appended here.
_TRACE = bool(os.environ.get("KBENCH_TRACE"))
EXEC_NS = []
TRACE_PATHS = []


def _run_spmd(nc, in_maps):
    res = run_bass_kernel_spmd(nc, in_maps, list(range(NCORES)),
                               trace=_TRACE,
                               trace_cores=list(range(NCORES)) if _TRACE
                               else None)
    if _TRACE:
        EXEC_NS.append(res.exec_time_ns)
        if res.instructions_and_trace is not None:
            TRACE_PATHS.append(res.instructions_and_trace[1])
    return res


def _bf16(a):
    import ml_dtypes
    return np.ascontiguousarray(np.asarray(a, np.float32)
                                .astype(ml_dtypes.bfloat16))


def _get_attn_nc(general_ln):
    key = ("attn", general_ln)
    if key not in _NC_CACHE:
        _NC_CACHE[key] = _build_attn(general_ln)
    return _NC_CACHE[key]


def _get_ffn_nc(general_ln, has_b1, alpha):
    key = ("ffn", general_ln, has_b1, float(alpha))
    if key not in _NC_CACHE:
        _NC_CACHE[key] = _build_ffn(general_ln, has_b1, float(alpha))
    return _NC_CACHE[key]


def _attn_weights(Wq, Wk, Wv, Wo):
    """Per-core weight arrays in the device layouts."""
    per_core = []
    for c in range(NCORES):
        hh = c % 2
        h0 = HPC * hh
        # [C, 512] -> [128, NCC, 512]
        def wlay(Wx):
            catw = np.concatenate([Wx[h] for h in range(h0, h0 + HPC)],
                                  axis=1)  # [C, 512]
            return _bf16(catw.reshape(NCC, 128, 512).transpose(1, 0, 2))
        wo = _bf16(Wo[hh * 512:(hh + 1) * 512].reshape(4, 128, C)
                   .transpose(1, 0, 2))
        per_core.append((wlay(Wq), wlay(Wk), wlay(Wv), wo))
    return per_core


def run_attn(x_flat, Wq, Wk, Wv, Wo, ln1_w, ln1_b):
    """Returns list of per-core partial projections [T, C] f32."""
    trivial = bool(np.all(ln1_w == 1.0) and np.all(ln1_b == 0.0))
    nc = _get_attn_nc(not trivial)
    wts = _attn_weights(Wq, Wk, Wv, Wo)
    in_maps = []
    for c in range(NCORES):
        b = c // 2
        wq, wk, wv, wo = wts[c]
        import ml_dtypes
        m = {"x": _bf16(x_flat[b * T:(b + 1) * T]),
             "wq": wq, "wk": wk, "wv": wv, "wo": wo,
             "ones": np.ones((1, 64), ml_dtypes.bfloat16)}
        if not trivial:
            m["lnw"] = np.asarray(ln1_w, np.float32)
            m["lnb"] = np.asarray(ln1_b, np.float32)
        in_maps.append(m)
    res = _run_spmd(nc, in_maps)
    return [res.results[c]["pout"] for c in range(NCORES)]


def run_ffn(x2_flat, W1, b1, W2, ln2_w, ln2_b, alpha):
    trivial = bool(np.all(ln2_w == 1.0) and np.all(ln2_b == 0.0))
    has_b1 = bool(np.any(b1 != 0.0))
    nc = _get_ffn_nc(not trivial, has_b1, alpha)
    w1l = _bf16(np.asarray(W1, np.float32)
                .reshape(NCC, 128, NHID, 128).transpose(1, 2, 0, 3))
    w2l = _bf16(np.asarray(W2, np.float32)
                .reshape(NHID, 128, C).transpose(1, 0, 2))
    in_maps = []
    for c in range(NCORES):
        rows = np.ascontiguousarray(x2_flat[RPC * c:RPC * (c + 1)])
        m = {"x2": rows, "x2l": _bf16(rows), "w1": w1l, "w2": w2l}
        if has_b1:
            m["b1"] = np.asarray(b1, np.float32)
        if not trivial:
            m["lnw"] = np.asarray(ln2_w, np.float32)
            m["lnb"] = np.asarray(ln2_b, np.float32)
        in_maps.append(m)
    res = _run_spmd(nc, in_maps)
    return np.concatenate(
        [res.results[c]["out"] for c in range(NCORES)], axis=0)


def kernel(x, ln1_w, ln1_b, Wk, Wq, Wv, Wo, bo, ln2_w, ln2_b, W1, b1,
           prelu_a, W2, b2):
    x = np.asarray(x, np.float32)
    x_flat = np.ascontiguousarray(x.reshape(B * T, C))
    alpha = float(np.asarray(prelu_a))

    parts = run_attn(x_flat, np.asarray(Wq, np.float32),
                     np.asarray(Wk, np.float32),
                     np.asarray(Wv, np.float32),
                     np.asarray(Wo, np.float32),
                     np.asarray(ln1_w, np.float32),
                     np.asarray(ln1_b, np.float32))
    # host reduction: x2 = x + partial_even + partial_odd (+ bo)
    x2 = np.empty_like(x_flat)
    for b in range(B):
        x2[b * T:(b + 1) * T] = (x_flat[b * T:(b + 1) * T]
                                 + parts[2 * b].astype(np.float32)
                                 + parts[2 * b + 1].astype(np.float32))
    bo = np.asarray(bo, np.float32)
    if np.any(bo != 0.0):
        x2 += bo
    out = run_ffn(x2, W1, np.asarray(b1, np.float32), W2,
                  np.asarray(ln2_w, np.float32),
                  np.asarray(ln2_b, np.float32), alpha)
    b2 = np.asarray(b2, np.float32)
    if np.any(b2 != 0.0):
        out = out + b2
    return out.reshape(B, T, C).astype(np.float32)
